# revision 1
# baseline (speedup 1.0000x reference)
"""Multi-head self-attention (B=8, S=2048, H=256, NH=8, HD=32) on 8 TRN2 cores.

Strategy: data-parallel over batch — each core computes full MHA for one
batch element; no collectives.

Per-core dataflow (all matmuls bf16 in / fp32 PSUM accum):
  - host ships x^T (features on partitions) so no on-device transpose
  - qkT:  q^T,k^T [feat, s] = w_qkv^T @ x — feature-major so each head's
    32 q/k features land on one 32-partition strip
  - scores^T per (head, key-tile): 4 heads computed concurrently via
    4x row-tiled PE (tile_position=(32i,0), K=32)
  - exp on ScalarE straight out of a 4-bank PSUM region ([128,2048] per
    ACTIVATE, scale=1/sqrt(HD) folded in); softmax max-subtraction is
    skipped (scores are O(1), no overflow risk in fp32)
  - ctx^T accumulated over key tiles with 2x column-tiled PE
    (tile_position=(0,0)/(0,64)); stationary v blocks carry a ones
    column so each 64-row tile yields [ctx_h(32) | rowsum(1) | pad]
  - ctx evicted unnormalized to SBUF staging (frees the accumulators for
    the next q-block); the 8 rowsums per q-block are gathered via DRAM,
    one batched VectorE reciprocal, partition-broadcast back via DRAM,
    and multiplied in from staging
  - out = ctxT^T @ w_out_perm + b_out; w_out rows are permuted/zero-padded
    on the host to match the ctxT slot layout
"""
import numpy as np
import ml_dtypes

import bass_rust
import concourse.bass as bass
import concourse.mybir as mybir
import concourse.tile as tile
from concourse.vector_clock import ScopedClock
from concourse.bass_utils import run_bass_kernel_spmd

BF16 = mybir.dt.bfloat16
F32 = mybir.dt.float32
NPBF16 = ml_dtypes.bfloat16

B, S, H = 8, 2048, 256
NH, HD = 8, 32
SCALE = 1.0 / float(np.sqrt(HD))
N_CORES = 8

# Set by a test harness to collect HW timing: {"trace": bool, "trace_cores": [...]}
TRACE_OPTS = {}
LAST_RESULT = None

def _legalize_sync_waits(nc):
    """The walrus build here rejects >1 sync wait per instruction, but Tile
    freely emits 2-3 (and the exit drain up to ~27).  Move excess waits onto
    same-engine NoOp carriers inserted immediately before the offending
    instruction — identical semantics (the engine blocks on each wait in
    program order)."""
    n = 0
    for f in nc.m.functions:
        for bb in f.blocks:
            insts = bb.instructions  # live list
            i = 0
            while i < len(insts):
                inst = insts[i]
                si = inst.sync_info
                if si is not None and len(si.on_wait) > 1:
                    waits = list(si.on_wait)
                    carriers = []
                    for w in waits[:-1]:
                        carriers.append(
                            mybir.InstNoOp(
                                name=f"{inst.name}-w{n}",
                                sync_info=mybir.SyncInfo(on_wait=[w], on_update=[]),
                                bass_nofuse=True,
                                engine=inst.engine,
                            )
                        )
                        n += 1
                    inst.sync_info = bass_rust.SyncInfo(
                        on_wait=waits[-1:], on_update=list(si.on_update)
                    )
                    insts[i:i] = carriers
                    i += len(carriers)
                i += 1
    return n


def _build_nc(legalize=True):
    nc = bass.Bass()
    xt = nc.dram_tensor("xt", [128, 2 * S], BF16, kind="ExternalInput")
    wqk = nc.dram_tensor("wqk", [128, 2 * 512], BF16, kind="ExternalInput")
    bv = nc.dram_tensor("bv", [1, 264], BF16, kind="ExternalInput")
    wv = nc.dram_tensor("wv", [128, 2 * 264], BF16, kind="ExternalInput")
    wo = nc.dram_tensor("wo", [128, 4 * 256], BF16, kind="ExternalInput")
    bqkc = nc.dram_tensor("bqkc", [128, 4], F32, kind="ExternalInput")
    ones = nc.dram_tensor("ones", [1, 512], BF16, kind="ExternalInput")
    zrow = nc.dram_tensor("zrow", [2, 2048], BF16, kind="ExternalInput")
    out = nc.dram_tensor("out", [S, H], F32, kind="ExternalOutput")
    # scratch for the partition-broadcast DMA roundtrip (SBUF APs cannot
    # have a zero partition step, DRAM APs can): one row per (qb, pair, side)
    rscr = nc.dram_tensor("rscr", [32, 512], F32)
    rscr2 = nc.dram_tensor("rscr2", [32, 512], F32)

    EXP = mybir.ActivationFunctionType.Exp

    with tile.TileContext(nc) as tc:
        with (
            tc.tile_pool(name="const", bufs=1) as const,
            tc.tile_pool(name="ev", bufs=8) as ev,
            tc.tile_pool(name="etp", bufs=4) as etp,
        ):
            xt_sb = const.tile([128, 2 * S], BF16, tag="xt")
            nc.sync.dma_start(out=xt_sb, in_=xt[:, :])
            wqk_sb = const.tile([128, 2 * 512], BF16, tag="wqk")
            nc.sync.dma_start(out=wqk_sb, in_=wqk[:, :])
            wv_sb = const.tile([128, 2 * 264], BF16, tag="wv")
            nc.sync.dma_start(out=wv_sb, in_=wv[:, :])
            wo_sb = const.tile([128, 4 * 256], BF16, tag="wo")
            nc.sync.dma_start(out=wo_sb, in_=wo[:, :])
            bv_sb = const.tile([1, 264], BF16, tag="bv")
            nc.sync.dma_start(out=bv_sb, in_=bv[:, :])
            ones1_sb = const.tile([1, 128], BF16, tag="ones1")
            nc.sync.dma_start(out=ones1_sb, in_=ones[0:1, 0:128])
            bqkc_sb = const.tile([128, 4], F32, tag="bqkc")
            nc.sync.dma_start(out=bqkc_sb, in_=bqkc[:, :])

            qT_sb = const.tile([128, 2 * S], BF16, tag="qT")
            kT_sb = const.tile([128, 2 * S], BF16, tag="kT")
            v_sb = const.tile([128, 16 * 264], BF16, tag="v")
            ctxT_sb = [
                const.tile([128, S], BF16, tag=f"ctxT{k}", name=f"ctxT{k}")
                for k in range(4)
            ]
            # rows 32:64 / 96:128 of each ctxT tile are never written by the
            # evictions but are contracted by the output matmul (against
            # zeroed w_out rows) — clear them via broadcast DMA so stale NaN
            # patterns can't poison the accumulation
            for k in range(4):
                if k == 0:
                    # row 32 of tile 0 is all-ones: paired with w_out_perm
                    # row 32 = b_out it adds the output bias for free
                    nc.sync.dma_start(out=ctxT_sb[0][32:33, :], in_=zrow[1:2, :])
                    nc.sync.dma_start(
                        out=ctxT_sb[0][33:64, :],
                        in_=zrow[0:1, :].to_broadcast((31, S)),
                    )
                else:
                    nc.sync.dma_start(
                        out=ctxT_sb[k][32:64, :],
                        in_=zrow[0:1, :].to_broadcast((32, S)),
                    )
                nc.sync.dma_start(
                    out=ctxT_sb[k][96:128, :],
                    in_=zrow[0:1, :].to_broadcast((32, S)),
                )

            # ---- phase 0: HAM warmup — ~6µs of dep-free back-to-back
            # matmuls so the PE clock gate opens (1.2 -> 2.4 GHz) before the
            # real work; garbage values land in a scratch PSUM bank that is
            # never read ----
            with tc.tile_pool(name="pp", bufs=4, space="PSUM") as pp:
                warm_sb = const.tile([128, 512], BF16, tag="warm")
                nc.vector.memset(warm_sb, 0.0)
                warm_ps = pp.tile([128, 512], F32, tag="pp")
                for _ in range(12):
                    nc.tensor.matmul(
                        out=warm_ps, lhsT=warm_sb[:, 0:128], rhs=warm_sb[:, :],
                        start=True, stop=True,
                    )

                # ---- phase 1: qT/kT [feature, s] = w_qkv^T @ x; bias folded
                #      into the eviction (per-partition, features-major) ----
                for t in range(4):  # feature tiles: q0,q1,k0,k1
                    for nb in range(4):  # s blocks of 512
                        ps = pp.tile([128, 512], F32, tag="pp")
                        for ks in range(2):
                            nc.tensor.matmul(
                                out=ps,
                                lhsT=wqk_sb[:, ks * 512 + t * 128 : ks * 512 + t * 128 + 128],
                                rhs=xt_sb[:, ks * S + nb * 512 : ks * S + nb * 512 + 512],
                                start=(ks == 0), stop=(ks == 1),
                            )
                        dst = (qT_sb if t < 2 else kT_sb)[
                            :, (t % 2) * S + nb * 512 : (t % 2) * S + nb * 512 + 512
                        ]
                        nc.vector.tensor_scalar_add(
                            out=dst, in0=ps, scalar1=bqkc_sb[:, t : t + 1]
                        )

                # ---- phase 2: v (natural layout, padded 64-wide head slots,
                #      ones column at j=32 for rowsums) ----
                for st in range(16):
                    ps = pp.tile([128, 264], F32, tag="ppv")
                    for ks in range(2):
                        nc.tensor.matmul(
                            out=ps,
                            lhsT=xt_sb[:, ks * S + st * 128 : ks * S + st * 128 + 128],
                            rhs=wv_sb[:, ks * 264 : ks * 264 + 264],
                            start=(ks == 0), stop=False,
                        )
                    # bias row also plants the rowsum ones-columns
                    nc.tensor.matmul(
                        out=ps,
                        lhsT=ones1_sb[0:1, 0:128],
                        rhs=bv_sb[0:1, 0:264],
                        start=False, stop=True,
                    )
                    dst = v_sb[:, st * 264 : st * 264 + 264]
                    nc.scalar.copy(out=dst, in_=ps)

            # ---- phase 3: attention, q-blocks of 512 ----
            with (
                tc.tile_pool(name="scp", bufs=2, space="PSUM") as scp,
                tc.tile_pool(name="cxp", bufs=4, space="PSUM") as cxp,
            ):
                for qb in range(4):
                    ctx_tiles = [
                        cxp.tile([128, 512], F32, tag="ctx", name=f"ctx_{qb}_{p}")
                        for p in range(4)
                    ]

                    def emit_ctx(g, kt, eT):
                        # ctx accumulation for (g, kt) — emitted one
                        # iteration late so these PE matmuls fill the window
                        # while ACT runs the *next* exp
                        for pi in range(2):
                            pair = g * 2 + pi
                            cps = ctx_tiles[pair]
                            vc = kt * 264 + pair * 66
                            nc.tensor.matmul(
                                out=cps[0:33, :],
                                lhsT=v_sb[:, vc : vc + 33],
                                rhs=eT[:, (2 * pi) * 512 : (2 * pi) * 512 + 512],
                                start=(kt == 0), stop=(kt == 15),
                                tile_position=(0, 0), skip_group_check=True,
                            )
                            nc.tensor.matmul(
                                out=cps[64:97, :],
                                lhsT=v_sb[:, vc + 33 : vc + 66],
                                rhs=eT[:, (2 * pi + 1) * 512 : (2 * pi + 1) * 512 + 512],
                                start=(kt == 0), stop=(kt == 15),
                                tile_position=(0, 64), skip_group_check=True,
                            )

                    pending = None
                    for kt in range(16):
                        for g in range(2):  # head groups of 4
                            eT = etp.tile([128, 2048], BF16, tag="eT")
                            # two half-groups in separate PSUM tiles: the
                            # half-B exp's WAR doesn't block half-A scores,
                            # so the next scores always overlap the running
                            # exp and ACT never waits on the PE
                            for half in range(2):
                                sc = scp.tile([128, 1024], F32, tag="sc",
                                              name=f"sc_{qb}_{kt}_{g}_{half}")
                                for i in (2 * half, 2 * half + 1):
                                    nc.tensor.matmul(
                                        out=sc[:, (i % 2) * 512 : (i % 2) * 512 + 512],
                                        lhsT=kT_sb[32 * i : 32 * i + 32,
                                                   g * S + kt * 128 : g * S + kt * 128 + 128],
                                        rhs=qT_sb[32 * i : 32 * i + 32,
                                                  g * S + qb * 512 : g * S + qb * 512 + 512],
                                        start=True, stop=True,
                                        tile_position=(32 * i, 0),
                                    )
                                nc.scalar.activation(
                                    out=eT[:, half * 1024 : half * 1024 + 1024],
                                    in_=sc,
                                    func=EXP, scale=SCALE,
                                )
                            if pending is not None:
                                emit_ctx(*pending)
                            pending = (g, kt, eT)
                    emit_ctx(*pending)
                    # Evict unnormalized ctx PSUM -> SBUF staging right away so
                    # the accumulator banks free for the next q-block, then
                    # normalize off the critical path: gather the 8 rowsum
                    # rows via DRAM into one [8,512] tile, one batched
                    # reciprocal (cost ~ free size only), broadcast back
                    # across partitions via DRAM, multiply from staging.
                    stages = []
                    for pair in range(4):
                        stg = ev.tile([128, 512], F32, tag="stg",
                                      name=f"stg_{qb}_{pair}")
                        nc.vector.tensor_copy(
                            out=stg[0:33, :], in_=ctx_tiles[pair][0:33, :]
                        )
                        nc.vector.tensor_copy(
                            out=stg[64:97, :], in_=ctx_tiles[pair][64:97, :]
                        )
                        stages.append(stg)
                        r0 = (qb * 4 + pair) * 2
                        nc.sync.dma_start(out=rscr[r0 : r0 + 1, :], in_=stg[32:33, :])
                        nc.sync.dma_start(out=rscr[r0 + 1 : r0 + 2, :], in_=stg[96:97, :])
                    rsg = ev.tile([8, 512], F32, tag="rsg")
                    nc.sync.dma_start(out=rsg, in_=rscr[qb * 8 : qb * 8 + 8, :])
                    nc.vector.reciprocal(out=rsg, in_=rsg)
                    nc.sync.dma_start(out=rscr2[qb * 8 : qb * 8 + 8, :], in_=rsg)
                    for pair in range(4):
                        stg = stages[pair]
                        rcb = ev.tile([128, 512], F32, tag="rcb",
                                      name=f"rcb_{qb}_{pair}")
                        r0 = (qb * 4 + pair) * 2
                        nc.sync.dma_start(
                            out=rcb[0:32, :],
                            in_=rscr2[r0 : r0 + 1, :].to_broadcast((32, 512)),
                        )
                        nc.sync.dma_start(
                            out=rcb[64:96, :],
                            in_=rscr2[r0 + 1 : r0 + 2, :].to_broadcast((32, 512)),
                        )
                        dst = ctxT_sb[pair]
                        nc.vector.tensor_mul(
                            out=dst[0:32, qb * 512 : qb * 512 + 512],
                            in0=stg[0:32, :], in1=rcb[0:32, :],
                        )
                        nc.vector.tensor_mul(
                            out=dst[64:96, qb * 512 : qb * 512 + 512],
                            in0=stg[64:96, :], in1=rcb[64:96, :],
                        )

            # ---- phase 4: out = ctxT^T @ w_out_perm + b_out ----
            with tc.tile_pool(name="op", bufs=4, space="PSUM") as op:
                for st in range(16):
                    ps = op.tile([128, 256], F32, tag="op")
                    for kk in range(4):
                        nc.tensor.matmul(
                            out=ps,
                            lhsT=ctxT_sb[kk][:, st * 128 : st * 128 + 128],
                            rhs=wo_sb[:, kk * 256 : kk * 256 + 256],
                            start=(kk == 0), stop=(kk == 3),
                        )
                    ot = ev.tile([128, 256], F32, tag="ot")
                    nc.vector.tensor_copy(out=ot, in_=ps)
                    nc.sync.dma_start(
                        out=out[st * 128 : st * 128 + 128, :], in_=ot
                    )
    if legalize:
        _legalize_sync_waits(nc)
    return nc


_NC_CACHE = None


def _get_nc():
    global _NC_CACHE
    if _NC_CACHE is None:
        _NC_CACHE = _build_nc()
    return _NC_CACHE


def _ks_layout(a, nk, cols):
    """[nk*128, cols] -> [128, nk*cols] with [p, k*cols+c] = a[k*128+p, c]."""
    return np.ascontiguousarray(
        a.reshape(nk, 128, cols).transpose(1, 0, 2).reshape(128, nk * cols)
    )


def _prep_in_maps(x, w_qkv, b_qkv, w_out, b_out):
    x = np.asarray(x, dtype=np.float32)
    w_qkv = np.asarray(w_qkv, dtype=np.float32)
    b_qkv = np.asarray(b_qkv, dtype=np.float32)
    w_out = np.asarray(w_out, dtype=np.float32)
    b_out = np.asarray(b_out, dtype=np.float32)

    # shared (per-core identical) weight layouts
    wqk_l = _ks_layout(w_qkv[:, : 2 * H], 2, 512).astype(NPBF16)

    # v weights: 64-wide slot per head: [v_h (32) | ones-col | 31 zero]
    # (the ones column itself is DMA'd on device; v bias is zero per spec)
    wpad = np.zeros((H, 264), np.float32)
    bvr = np.zeros((1, 264), np.float32)
    for h in range(NH):
        c0 = h * 33
        wpad[:, c0 : c0 + 32] = w_qkv[:, 2 * H + h * HD : 2 * H + (h + 1) * HD]
        bvr[0, c0 : c0 + 32] = b_qkv[2 * H + h * HD : 2 * H + (h + 1) * HD]
        bvr[0, c0 + 32] = 1.0  # ones column -> rowsum row
    wv_l = _ks_layout(wpad, 2, 264).astype(NPBF16)


    # w_out rows permuted into the ctxT slot layout (zeros in pad slots)
    wo_perm = np.zeros((512, H), np.float32)
    for pair in range(4):
        for side in range(2):
            h = 2 * pair + side
            r0 = pair * 128 + side * 64
            wo_perm[r0 : r0 + 32, :] = w_out[h * HD : (h + 1) * HD, :]
    wo_perm[32, :] = b_out  # multiplied by the ctxT[0] ones row
    wo_l = _ks_layout(wo_perm, 4, 256).astype(NPBF16)

    shared = {
        "wqk": wqk_l,
        "wv": wv_l,
        "bv": bvr.astype(NPBF16),
        "wo": wo_l,
        "bqkc": np.ascontiguousarray(
            b_qkv[: 2 * H].astype(np.float32).reshape(4, 128).T
        ),
        "ones": np.ones((1, 512), NPBF16),
        "zrow": np.concatenate([np.zeros((1, 2048), NPBF16), np.ones((1, 2048), NPBF16)]),
    }
    in_maps = []
    for b in range(B):
        xt = _ks_layout(np.ascontiguousarray(x[b].T), 2, S).astype(NPBF16)
        in_maps.append({"xt": xt, **shared})
    return in_maps


def kernel(x, w_qkv, b_qkv, w_out, b_out):
    in_maps = _prep_in_maps(x, w_qkv, b_qkv, w_out, b_out)
    nc = _get_nc()
    res = run_bass_kernel_spmd(nc, in_maps, list(range(N_CORES)), **TRACE_OPTS)
    global LAST_RESULT
    LAST_RESULT = res
    return np.stack([res.results[b]["out"] for b in range(B)], axis=0)



# revision 6
# speedup vs baseline: 1.0387x; 1.0387x over previous
"""Multi-head self-attention (B=8, S=2048, H=256, NH=8, HD=32) on 8 TRN2 cores.

Strategy: data-parallel over batch — each core computes full MHA for one
batch element; no collectives.

Per-core dataflow (all matmuls bf16 in / fp32 PSUM accum):
  - host ships x^T (features on partitions) so no on-device transpose
  - qkT:  q^T,k^T [feat, s] = w_qkv^T @ x — feature-major so each head's
    32 q/k features land on one 32-partition strip; bias folded into the
    eviction (split between ScalarE and VectorE, both idle here)
  - scores^T per (head, key-tile): 4 heads computed concurrently via
    4x row-tiled PE (tile_position=(32i,0), K=32)
  - softmax exp is the kernel bottleneck (NH*S*S = 33.5M elements/core,
    and exp natively runs only on ScalarE at 1 col/cycle).  The exp work
    is therefore SPLIT between two engines:
      * ScalarE share: ACTIVATE(Exp, scale=1/sqrt(HD)) from PSUM
      * VectorE share: one TENSOR_SCALAR computing the Schraudolph bit
        trick — i16 = trunc(score * (128*log2(e)/sqrt(HD)) + (127*128-C))
        written through an int16 bitcast of the bf16 eT tile.  The int16
        bit pattern IS bf16(exp(score/sqrt(HD))) up to ~2% sawtooth error
        which largely cancels under softmax renormalization (validated:
        global rel err 0.008 vs 0.005 for exact exp).
    The 16-slot assignment pattern interleaves the engines ~9:7 to
    balance ScalarE@1.2GHz against VectorE@0.96GHz + its other work.
  - softmax max-subtraction is skipped (scores are O(1), fp32 PSUM)
  - ctx^T accumulated over key tiles with 4x column-tiled PE
    (tile_position=(0,32h)): all 4 heads of a group land fully packed in
    ONE [128,512] PSUM bank; rowsums likewise accumulate as separate
    M=1 column-tiled matmuls (lhsT = ones column) into a second bank at
    partitions {0,32,64,96}
  - normalization off the critical path: ctx/rowsum banks evicted by
    single full-tile engine copies into [128,512] fp32 staging (frees
    the accumulators), rowsum rows shipped via DRAM into one [128,32]
    tile, one VectorE reciprocal, scattered back and partition-broadcast
    via DRAM, then 2 full-tile [128,512] multiplies write the normalized
    bf16 ctxT.  The VectorE pieces are emitted a few iterations into the
    NEXT q-block so the DMA round-trip never head-of-line-blocks the
    VectorE exp stream.
  - out = ctxT^T @ w_out + b_out: ctxT is fully packed (2 tiles of
    4 heads x 32 rows), w_out needs no permutation, bias comes from a
    K=1 ones-row matmul, and the result goes PSUM -> SBUF -> DRAM.
"""
import math

import numpy as np
import ml_dtypes

import bass_rust
import concourse.bass as bass
import concourse.mybir as mybir
import concourse.tile as tile
from concourse.vector_clock import ScopedClock
from concourse.bass_utils import run_bass_kernel_spmd

BF16 = mybir.dt.bfloat16
F32 = mybir.dt.float32
NPBF16 = ml_dtypes.bfloat16

B, S, H = 8, 2048, 256
NH, HD = 8, 32
SCALE = 1.0 / float(np.sqrt(HD))
N_CORES = 8

# Schraudolph-exp constants for the VectorE share: bf16 bit pattern of
# exp(SCALE*x) ~= trunc(x * A16 + B16) interpreted as int16.
A16 = SCALE * 128.0 / math.log(2.0)
B16 = 127.0 * 128.0 - 6.0

# Which of each 16 consecutive (qb,kt,g,half) exp tiles go to VectorE
# (7 of 16; the rest go to ScalarE).  Evenly interleaved.
DVE_SLOTS = frozenset(i for i in range(16) if (i * 7) % 16 < 7)

# Set by a test harness to collect HW timing: {"trace": bool, "trace_cores": [...]}
TRACE_OPTS = {}
LAST_RESULT = None

def _legalize_sync_waits(nc):
    """The walrus build here rejects >1 sync wait per instruction, but Tile
    freely emits 2-3 (and the exit drain up to ~27).  Move excess waits onto
    same-engine NoOp carriers inserted immediately before the offending
    instruction — identical semantics (the engine blocks on each wait in
    program order)."""
    n = 0
    for f in nc.m.functions:
        for bb in f.blocks:
            insts = bb.instructions  # live list
            i = 0
            while i < len(insts):
                inst = insts[i]
                si = inst.sync_info
                if si is not None and len(si.on_wait) > 1:
                    waits = list(si.on_wait)
                    carriers = []
                    for w in waits[:-1]:
                        carriers.append(
                            mybir.InstNoOp(
                                name=f"{inst.name}-w{n}",
                                sync_info=mybir.SyncInfo(on_wait=[w], on_update=[]),
                                bass_nofuse=True,
                                engine=inst.engine,
                            )
                        )
                        n += 1
                    inst.sync_info = bass_rust.SyncInfo(
                        on_wait=waits[-1:], on_update=list(si.on_update)
                    )
                    insts[i:i] = carriers
                    i += len(carriers)
                i += 1
    return n


def _build_nc(legalize=True):
    nc = bass.Bass()
    xt = nc.dram_tensor("xt", [128, 2 * S], BF16, kind="ExternalInput")
    wqk = nc.dram_tensor("wqk", [128, 2 * 512], BF16, kind="ExternalInput")
    bv = nc.dram_tensor("bv", [1, 256], BF16, kind="ExternalInput")
    wv = nc.dram_tensor("wv", [128, 2 * 256], BF16, kind="ExternalInput")
    wo = nc.dram_tensor("wo", [128, 2 * 256], BF16, kind="ExternalInput")
    bo = nc.dram_tensor("bo", [1, 256], BF16, kind="ExternalInput")
    bqkc = nc.dram_tensor("bqkc", [128, 4], F32, kind="ExternalInput")
    ones = nc.dram_tensor("ones", [1, 512], BF16, kind="ExternalInput")
    out = nc.dram_tensor("out", [S, H], F32, kind="ExternalOutput")
    # scratch for the rowsum gather / reciprocal-broadcast DMA roundtrips
    # (SBUF APs cannot have a zero partition step, DRAM APs can)
    rscr = nc.dram_tensor("rscr", [32, 512], F32)
    rscr2 = nc.dram_tensor("rscr2", [32, 512], F32)

    EXP = mybir.ActivationFunctionType.Exp
    IDN = mybir.ActivationFunctionType.Identity
    MUL = mybir.AluOpType.mult
    ADD = mybir.AluOpType.add

    with tile.TileContext(nc) as tc:
        with (
            tc.tile_pool(name="const", bufs=1) as const,
            tc.tile_pool(name="etp", bufs=4) as etp,
            tc.tile_pool(name="nrm", bufs=2) as nrm,
        ):
            xt_sb = const.tile([128, 2 * S], BF16, tag="xt")
            nc.sync.dma_start(out=xt_sb, in_=xt[:, :])
            wqk_sb = const.tile([128, 2 * 512], BF16, tag="wqk")
            nc.sync.dma_start(out=wqk_sb, in_=wqk[:, :])
            wv_sb = const.tile([128, 2 * 256], BF16, tag="wv")
            nc.sync.dma_start(out=wv_sb, in_=wv[:, :])
            wo_sb = const.tile([128, 2 * 256], BF16, tag="wo")
            nc.sync.dma_start(out=wo_sb, in_=wo[:, :])
            bv_sb = const.tile([1, 256], BF16, tag="bv")
            nc.sync.dma_start(out=bv_sb, in_=bv[:, :])
            bo_sb = const.tile([1, 256], BF16, tag="bo")
            nc.sync.dma_start(out=bo_sb, in_=bo[:, :])
            ones1_sb = const.tile([1, 128], BF16, tag="ones1")
            nc.sync.dma_start(out=ones1_sb, in_=ones[0:1, 0:128])
            bqkc_sb = const.tile([128, 4], F32, tag="bqkc")
            nc.sync.dma_start(out=bqkc_sb, in_=bqkc[:, :])
            # ones column for the rowsum matmuls (K=128, M=1)
            onec_sb = const.tile([128, 1], BF16, tag="onec")
            nc.vector.memset(onec_sb, 1.0)

            qT_sb = const.tile([128, 2 * S], BF16, tag="qT")
            kT_sb = const.tile([128, 2 * S], BF16, tag="kT")
            v_sb = const.tile([128, 16 * 256], BF16, tag="v")
            # fully-packed normalized ctx^T: tile g holds heads 4g..4g+3,
            # rows h*32..h*32+32 = head (4g+h) features, cols = q positions
            ctxT_sb = [
                const.tile([128, S], BF16, tag=f"ctxT{g}", name=f"ctxT{g}")
                for g in range(2)
            ]

            # ---- phase 0: HAM warmup — ~6µs of dep-free back-to-back
            # matmuls so the PE clock gate opens (1.2 -> 2.4 GHz) before the
            # real work; garbage values land in a scratch PSUM bank that is
            # never read.  A dummy exp on ScalarE pulls the ~2.7µs ACT
            # table load off the critical path too. ----
            with tc.tile_pool(name="pp", bufs=4, space="PSUM") as pp:
                warm_sb = const.tile([128, 512], BF16, tag="warm")
                nc.vector.memset(warm_sb, 0.0)
                dume_sb = const.tile([1, 16], BF16, tag="dume")
                nc.scalar.activation(
                    out=dume_sb, in_=warm_sb[0:1, 0:16], func=EXP, scale=SCALE
                )
                warm_ps = pp.tile([128, 512], F32, tag="pp")
                for _ in range(12):
                    nc.tensor.matmul(
                        out=warm_ps, lhsT=warm_sb[:, 0:128], rhs=warm_sb[:, :],
                        start=True, stop=True,
                    )

                # ---- phase 1: qT/kT [feature, s] = w_qkv^T @ x; bias folded
                #      into the eviction (per-partition, features-major),
                #      evictions alternating ScalarE/VectorE ----
                for t in range(4):  # feature tiles: q0,q1,k0,k1
                    for nb in range(4):  # s blocks of 512
                        ps = pp.tile([128, 512], F32, tag="pp")
                        for ks in range(2):
                            nc.tensor.matmul(
                                out=ps,
                                lhsT=wqk_sb[:, ks * 512 + t * 128 : ks * 512 + t * 128 + 128],
                                rhs=xt_sb[:, ks * S + nb * 512 : ks * S + nb * 512 + 512],
                                start=(ks == 0), stop=(ks == 1),
                            )
                        dst = (qT_sb if t < 2 else kT_sb)[
                            :, (t % 2) * S + nb * 512 : (t % 2) * S + nb * 512 + 512
                        ]
                        if (t * 4 + nb) % 2 == 0:
                            nc.scalar.activation(
                                out=dst, in_=ps, func=IDN,
                                bias=bqkc_sb[:, t : t + 1], scale=1.0,
                            )
                        else:
                            nc.vector.tensor_scalar_add(
                                out=dst, in0=ps, scalar1=bqkc_sb[:, t : t + 1]
                            )

                # ---- phase 2: v (natural layout, 32-wide head slots),
                #      evictions alternating ScalarE/VectorE ----
                for st in range(16):
                    ps = pp.tile([128, 256], F32, tag="ppv")
                    for ks in range(2):
                        nc.tensor.matmul(
                            out=ps,
                            lhsT=xt_sb[:, ks * S + st * 128 : ks * S + st * 128 + 128],
                            rhs=wv_sb[:, ks * 256 : ks * 256 + 256],
                            start=(ks == 0), stop=False,
                        )
                    nc.tensor.matmul(
                        out=ps,
                        lhsT=ones1_sb[0:1, 0:128],
                        rhs=bv_sb[0:1, 0:256],
                        start=False, stop=True,
                    )
                    dst = v_sb[:, st * 256 : st * 256 + 256]
                    if st % 2 == 0:
                        nc.scalar.copy(out=dst, in_=ps)
                    else:
                        nc.vector.tensor_copy(out=dst, in_=ps)

            # ---- phase 3: attention, q-blocks of 512 ----
            hidx = 0  # global exp-tile counter for the engine split

            def evict_qb(qb, ctx_ps, rs_ps):
                # engine copies out of PSUM (frees the accumulator banks),
                # then ship the 8 rowsum rows to DRAM and gather them back
                # as one [128, 32] tile
                stg, rss = [], []
                for g in range(2):
                    sg = nrm.tile([128, 512], F32, tag=f"stg{g}", name=f"stg_{qb}_{g}")
                    nc.scalar.copy(out=sg, in_=ctx_ps[g])
                    stg.append(sg)
                    rg = nrm.tile([128, 512], F32, tag=f"rss{g}", name=f"rss_{qb}_{g}")
                    if g == 0:
                        nc.scalar.copy(out=rg, in_=rs_ps[g])
                    else:
                        nc.vector.tensor_copy(out=rg, in_=rs_ps[g])
                    rss.append(rg)
                for g in range(2):
                    for hh in range(4):
                        r = qb * 8 + g * 4 + hh
                        nc.sync.dma_start(
                            out=rscr[r : r + 1, :],
                            in_=rss[g][hh * 32 : hh * 32 + 1, :],
                        )
                rsg = nrm.tile([128, 32], F32, tag="rsg", name=f"rsg_{qb}")
                nc.sync.dma_start(
                    out=rsg,
                    in_=rscr[qb * 8 : qb * 8 + 8, :].rearrange("r (c k) -> (r c) k", k=32),
                )
                return stg, rsg

            def norm_a(qb, stg, rsg):
                # reciprocal + scatter + partition-broadcast roundtrip
                rsgi = nrm.tile([128, 32], F32, tag="rsgi", name=f"rsgi_{qb}")
                nc.vector.reciprocal(out=rsgi, in_=rsg)
                nc.sync.dma_start(
                    out=rscr2[qb * 8 : qb * 8 + 8, :].rearrange("r (c k) -> (r c) k", k=32),
                    in_=rsgi,
                )
                rcb = []
                for g in range(2):
                    rc = nrm.tile([128, 512], F32, tag=f"rcb{g}", name=f"rcb_{qb}_{g}")
                    for hh in range(4):
                        r = qb * 8 + g * 4 + hh
                        nc.sync.dma_start(
                            out=rc[hh * 32 : hh * 32 + 32, :],
                            in_=rscr2[r : r + 1, :].to_broadcast((32, 512)),
                        )
                    rcb.append(rc)
                return rcb

            def norm_b(qb, stg, rcb):
                for g in range(2):
                    nc.vector.tensor_mul(
                        out=ctxT_sb[g][:, qb * 512 : qb * 512 + 512],
                        in0=stg[g], in1=rcb[g],
                    )

            with (
                tc.tile_pool(name="scp", bufs=2, space="PSUM") as scp,
                tc.tile_pool(name="cxp", bufs=1, space="PSUM") as cxp,
            ):
                pending_norm = None  # (qb, stg, rsg) awaiting recip+mul
                for qb in range(4):
                    ctx_ps = [
                        cxp.tile([128, 512], F32, tag=f"ctx{g}", name=f"ctx_{qb}_{g}")
                        for g in range(2)
                    ]
                    rs_ps = [
                        cxp.tile([128, 512], F32, tag=f"rs{g}", name=f"rs_{qb}_{g}")
                        for g in range(2)
                    ]

                    def emit_ctx(g, kt, eT):
                        # ctx + rowsum accumulation for (g, kt) — emitted one
                        # iteration late so these PE matmuls fill the window
                        # while ACT/DVE run the *next* exp.  4 heads packed
                        # via 4x column tiling; rowsums are M=1 matmuls
                        # against a ones column at the same positions.
                        for hh in range(4):
                            nc.tensor.matmul(
                                out=ctx_ps[g][hh * 32 : hh * 32 + 32, :],
                                lhsT=v_sb[:, kt * 256 + (g * 4 + hh) * 32 : kt * 256 + (g * 4 + hh) * 32 + 32],
                                rhs=eT[:, hh * 512 : hh * 512 + 512],
                                start=(kt == 0), stop=(kt == 15),
                                tile_position=(0, hh * 32), skip_group_check=True,
                            )
                        for hh in range(4):
                            nc.tensor.matmul(
                                out=rs_ps[g][hh * 32 : hh * 32 + 1, :],
                                lhsT=onec_sb[:, 0:1],
                                rhs=eT[:, hh * 512 : hh * 512 + 512],
                                start=(kt == 0), stop=(kt == 15),
                                tile_position=(0, hh * 32), skip_group_check=True,
                            )

                    pending = None
                    for kt in range(16):
                        for g in range(2):  # head groups of 4
                            # normalize for the previous q-block, emitted here
                            # so its DMA roundtrips overlap exp work instead
                            # of blocking the VectorE queue
                            if pending_norm is not None and g == 0:
                                pqb, pstg, prsg = pending_norm
                                if kt == 1:
                                    pending_norm = (pqb, pstg, norm_a(pqb, pstg, prsg))
                                elif kt == 3:
                                    norm_b(pqb, pstg, pending_norm[2])
                                    pending_norm = None
                            eT = etp.tile([128, 2048], BF16, tag="eT")
                            # two half-groups in separate PSUM tiles: the
                            # half-B exp's WAR doesn't block half-A scores,
                            # so the next scores always overlap the running
                            # exp and the exp engines never wait on the PE
                            for half in range(2):
                                sc = scp.tile([128, 1024], F32, tag="sc",
                                              name=f"sc_{qb}_{kt}_{g}_{half}")
                                for i in (2 * half, 2 * half + 1):
                                    nc.tensor.matmul(
                                        out=sc[:, (i % 2) * 512 : (i % 2) * 512 + 512],
                                        lhsT=kT_sb[32 * i : 32 * i + 32,
                                                   g * S + kt * 128 : g * S + kt * 128 + 128],
                                        rhs=qT_sb[32 * i : 32 * i + 32,
                                                  g * S + qb * 512 : g * S + qb * 512 + 512],
                                        start=True, stop=True,
                                        tile_position=(32 * i, 0),
                                    )
                                edst = eT[:, half * 1024 : half * 1024 + 1024]
                                if (hidx % 16) in DVE_SLOTS:
                                    nc.vector.tensor_scalar(
                                        out=edst.bitcast(mybir.dt.int16),
                                        in0=sc, scalar1=A16, scalar2=B16,
                                        op0=MUL, op1=ADD,
                                    )
                                else:
                                    nc.scalar.activation(
                                        out=edst, in_=sc, func=EXP, scale=SCALE,
                                    )
                                hidx += 1
                            if pending is not None:
                                emit_ctx(*pending)
                            pending = (g, kt, eT)
                    emit_ctx(*pending)
                    stg, rsg = evict_qb(qb, ctx_ps, rs_ps)
                    pending_norm = (qb, stg, rsg)

                # tail: normalize the last q-block
                pqb, pstg, prsg = pending_norm
                rcb = norm_a(pqb, pstg, prsg)
                norm_b(pqb, pstg, rcb)

            # ---- phase 4: out = ctxT^T @ w_out + b_out (K=1 ones-row
            #      matmul adds the bias) ----
            with (
                tc.tile_pool(name="op", bufs=4, space="PSUM") as op,
                tc.tile_pool(name="ev", bufs=4) as ev,
            ):
                for st in range(16):
                    ps = op.tile([128, 256], F32, tag="op")
                    nc.tensor.matmul(
                        out=ps, lhsT=ones1_sb[0:1, 0:128], rhs=bo_sb[0:1, :],
                        start=True, stop=False,
                    )
                    for g in range(2):
                        nc.tensor.matmul(
                            out=ps,
                            lhsT=ctxT_sb[g][:, st * 128 : st * 128 + 128],
                            rhs=wo_sb[:, g * 256 : g * 256 + 256],
                            start=False, stop=(g == 1),
                        )
                    ot = ev.tile([128, 256], F32, tag="ot")
                    if st % 2 == 0:
                        nc.scalar.copy(out=ot, in_=ps)
                    else:
                        nc.vector.tensor_copy(out=ot, in_=ps)
                    nc.sync.dma_start(
                        out=out[st * 128 : st * 128 + 128, :], in_=ot
                    )
    if legalize:
        _legalize_sync_waits(nc)
    return nc


_NC_CACHE = None


def _get_nc():
    global _NC_CACHE
    if _NC_CACHE is None:
        _NC_CACHE = _build_nc()
    return _NC_CACHE


def _ks_layout(a, nk, cols):
    """[nk*128, cols] -> [128, nk*cols] with [p, k*cols+c] = a[k*128+p, c]."""
    return np.ascontiguousarray(
        a.reshape(nk, 128, cols).transpose(1, 0, 2).reshape(128, nk * cols)
    )


def _prep_in_maps(x, w_qkv, b_qkv, w_out, b_out):
    x = np.asarray(x, dtype=np.float32)
    w_qkv = np.asarray(w_qkv, dtype=np.float32)
    b_qkv = np.asarray(b_qkv, dtype=np.float32)
    w_out = np.asarray(w_out, dtype=np.float32)
    b_out = np.asarray(b_out, dtype=np.float32)

    # shared (per-core identical) weight layouts
    wqk_l = _ks_layout(w_qkv[:, : 2 * H], 2, 512).astype(NPBF16)
    # v weights in natural head order (32-wide slots)
    wv_l = _ks_layout(w_qkv[:, 2 * H :], 2, 256).astype(NPBF16)
    # out projection: ctxT is packed [(head h)*32 + d] so w_out needs no
    # permutation, only the K-split layout
    wo_l = _ks_layout(w_out, 2, 256).astype(NPBF16)

    shared = {
        "wqk": wqk_l,
        "wv": wv_l,
        "bv": b_qkv[2 * H :].reshape(1, H).astype(NPBF16),
        "wo": wo_l,
        "bo": b_out.reshape(1, H).astype(NPBF16),
        "bqkc": np.ascontiguousarray(
            b_qkv[: 2 * H].astype(np.float32).reshape(4, 128).T
        ),
        "ones": np.ones((1, 512), NPBF16),
    }
    in_maps = []
    for b in range(B):
        xt = _ks_layout(np.ascontiguousarray(x[b].T), 2, S).astype(NPBF16)
        in_maps.append({"xt": xt, **shared})
    return in_maps


def kernel(x, w_qkv, b_qkv, w_out, b_out):
    in_maps = _prep_in_maps(x, w_qkv, b_qkv, w_out, b_out)
    nc = _get_nc()
    res = run_bass_kernel_spmd(nc, in_maps, list(range(N_CORES)), **TRACE_OPTS)
    global LAST_RESULT
    LAST_RESULT = res
    return np.stack([res.results[b]["out"] for b in range(B)], axis=0)


# revision 9
# speedup vs baseline: 1.0826x; 1.0422x over previous
"""Multi-head self-attention (B=8, S=2048, H=256, NH=8, HD=32) on 8 TRN2 cores.

Strategy: data-parallel over batch — each core computes full MHA for one
batch element; no collectives.

Per-core dataflow (all matmuls bf16 in / fp32 PSUM accum):
  - host ships x^T (features on partitions) so no on-device transpose
  - qkT:  q^T,k^T [feat, s] = w_qkv^T @ x — feature-major so each head's
    32 q/k features land on one 32-partition strip; bias folded into the
    eviction (split between ScalarE and VectorE, both idle here)
  - scores^T per (head, key-tile): 4 heads computed concurrently via
    4x row-tiled PE (tile_position=(32i,0), K=32)
  - softmax exp is the kernel bottleneck (NH*S*S = 33.5M elements/core,
    and exp natively runs only on ScalarE at 1 col/cycle).  The exp work
    is therefore SPLIT between two engines:
      * ScalarE share: ACTIVATE(Exp, scale=1/sqrt(HD)) from PSUM
      * VectorE share: one TENSOR_SCALAR computing the Schraudolph bit
        trick — i16 = trunc(score * (128*log2(e)/sqrt(HD)) + (127*128-C))
        written through an int16 bitcast of the bf16 eT tile.  The int16
        bit pattern IS bf16(exp(score/sqrt(HD))) up to ~2% sawtooth error
        which largely cancels under softmax renormalization (validated:
        global rel err 0.008 vs 0.005 for exact exp).
    The 16-slot assignment pattern interleaves the engines ~9:7 to
    balance ScalarE@1.2GHz against VectorE@0.96GHz + its other work.
  - softmax max-subtraction is skipped (scores are O(1), fp32 PSUM)
  - ctx^T accumulated over key tiles with 4x column-tiled PE
    (tile_position=(0,32h)): all 4 heads of a group land fully packed in
    ONE [128,512] PSUM bank; rowsums likewise accumulate as separate
    M=1 column-tiled matmuls (lhsT = ones column) into a second bank at
    partitions {0,32,64,96}
  - normalization off the critical path: ctx/rowsum banks evicted by
    single full-tile engine copies into [128,512] fp32 staging (frees
    the accumulators), rowsum rows shipped via DRAM into one [128,32]
    tile, one VectorE reciprocal, scattered back and partition-broadcast
    via DRAM, then 2 full-tile [128,512] multiplies write the normalized
    bf16 ctxT.  The VectorE pieces are emitted a few iterations into the
    NEXT q-block so the DMA round-trip never head-of-line-blocks the
    VectorE exp stream.
  - out = ctxT^T @ w_out + b_out: ctxT is fully packed (2 tiles of
    4 heads x 32 rows), w_out needs no permutation, bias comes from a
    K=1 ones-row matmul, and the result goes PSUM -> SBUF -> DRAM.
"""
import math

import numpy as np
import ml_dtypes

import bass_rust
import concourse.bass as bass
import concourse.mybir as mybir
import concourse.tile as tile
from concourse.vector_clock import ScopedClock
from concourse.bass_utils import run_bass_kernel_spmd

BF16 = mybir.dt.bfloat16
F32 = mybir.dt.float32
NPBF16 = ml_dtypes.bfloat16

B, S, H = 8, 2048, 256
NH, HD = 8, 32
SCALE = 1.0 / float(np.sqrt(HD))
N_CORES = 8

# Schraudolph-exp constants for the VectorE share: bf16 bit pattern of
# exp(SCALE*x) ~= trunc(x * A16 + B16) interpreted as int16.
A16 = SCALE * 128.0 / math.log(2.0)
B16 = 127.0 * 128.0 - 6.0

# Which of each 16 consecutive (qb,kt,g,half) exp tiles go to VectorE
# (7 of 16; the rest go to ScalarE).  Evenly interleaved.
DVE_SLOTS = frozenset(i for i in range(16) if (i * 7) % 16 < 7)

# Set by a test harness to collect HW timing: {"trace": bool, "trace_cores": [...]}
TRACE_OPTS = {}
LAST_RESULT = None

def _legalize_sync_waits(nc):
    """The walrus build here rejects >1 sync wait per instruction, but Tile
    freely emits 2-3 (and the exit drain up to ~27).  Move excess waits onto
    same-engine NoOp carriers inserted immediately before the offending
    instruction — identical semantics (the engine blocks on each wait in
    program order)."""
    n = 0
    for f in nc.m.functions:
        for bb in f.blocks:
            insts = bb.instructions  # live list
            i = 0
            while i < len(insts):
                inst = insts[i]
                si = inst.sync_info
                if si is not None and len(si.on_wait) > 1:
                    waits = list(si.on_wait)
                    carriers = []
                    for w in waits[:-1]:
                        carriers.append(
                            mybir.InstNoOp(
                                name=f"{inst.name}-w{n}",
                                sync_info=mybir.SyncInfo(on_wait=[w], on_update=[]),
                                bass_nofuse=True,
                                engine=inst.engine,
                            )
                        )
                        n += 1
                    inst.sync_info = bass_rust.SyncInfo(
                        on_wait=waits[-1:], on_update=list(si.on_update)
                    )
                    insts[i:i] = carriers
                    i += len(carriers)
                i += 1
    return n


def _build_nc(legalize=True):
    nc = bass.Bass()
    xt = nc.dram_tensor("xt", [128, 2 * S], BF16, kind="ExternalInput")
    wqk = nc.dram_tensor("wqk", [128, 2 * 512], BF16, kind="ExternalInput")
    bv = nc.dram_tensor("bv", [1, 256], BF16, kind="ExternalInput")
    wv = nc.dram_tensor("wv", [128, 2 * 256], BF16, kind="ExternalInput")
    wo = nc.dram_tensor("wo", [128, 2 * 256], BF16, kind="ExternalInput")
    bo = nc.dram_tensor("bo", [1, 256], BF16, kind="ExternalInput")
    bqkc = nc.dram_tensor("bqkc", [128, 4], F32, kind="ExternalInput")
    ones = nc.dram_tensor("ones", [1, 512], BF16, kind="ExternalInput")
    out = nc.dram_tensor("out", [S, H], F32, kind="ExternalOutput")
    # scratch for the rowsum gather / reciprocal-broadcast DMA roundtrips
    # (SBUF APs cannot have a zero partition step, DRAM APs can)
    rscr = nc.dram_tensor("rscr", [32, 512], F32)
    rscr2 = nc.dram_tensor("rscr2", [32, 512], F32)

    EXP = mybir.ActivationFunctionType.Exp
    IDN = mybir.ActivationFunctionType.Identity
    MUL = mybir.AluOpType.mult
    ADD = mybir.AluOpType.add

    with tile.TileContext(nc) as tc:
        with (
            tc.tile_pool(name="const", bufs=1) as const,
            tc.tile_pool(name="etp", bufs=4) as etp,
            tc.tile_pool(name="nrm", bufs=2) as nrm,
        ):
            xt_sb = const.tile([128, 2 * S], BF16, tag="xt")
            nc.sync.dma_start(out=xt_sb, in_=xt[:, :])
            wqk_sb = const.tile([128, 2 * 512], BF16, tag="wqk")
            nc.sync.dma_start(out=wqk_sb, in_=wqk[:, :])
            wv_sb = const.tile([128, 2 * 256], BF16, tag="wv")
            nc.sync.dma_start(out=wv_sb, in_=wv[:, :])
            wo_sb = const.tile([128, 2 * 256], BF16, tag="wo")
            nc.sync.dma_start(out=wo_sb, in_=wo[:, :])
            bv_sb = const.tile([1, 256], BF16, tag="bv")
            nc.sync.dma_start(out=bv_sb, in_=bv[:, :])
            bo_sb = const.tile([1, 256], BF16, tag="bo")
            nc.sync.dma_start(out=bo_sb, in_=bo[:, :])
            ones1_sb = const.tile([1, 128], BF16, tag="ones1")
            nc.sync.dma_start(out=ones1_sb, in_=ones[0:1, 0:128])
            bqkc_sb = const.tile([128, 4], F32, tag="bqkc")
            nc.sync.dma_start(out=bqkc_sb, in_=bqkc[:, :])
            # rowsum selector columns (K=128, M=2): group 0 contracts
            # against [1|0] (sum lands in row +0), group 1 against [0|1]
            # (row +1) — both groups share one PSUM rowsum bank
            onesel_sb = const.tile([128, 4], BF16, tag="onesel")
            nc.vector.memset(onesel_sb[:, 0:1], 1.0)
            nc.vector.memset(onesel_sb[:, 1:3], 0.0)
            nc.vector.memset(onesel_sb[:, 3:4], 1.0)
            # ones column for warm-keeper filler matmuls
            onec_sb = const.tile([128, 1], BF16, tag="onec")
            nc.vector.memset(onec_sb, 1.0)

            qT_sb = const.tile([128, 2 * S], BF16, tag="qT")
            kT_sb = const.tile([128, 2 * S], BF16, tag="kT")
            v_sb = const.tile([128, 16 * 256], BF16, tag="v")
            # fully-packed normalized ctx^T: tile g holds heads 4g..4g+3,
            # rows h*32..h*32+32 = head (4g+h) features, cols = q positions
            ctxT_sb = [
                const.tile([128, S], BF16, tag=f"ctxT{g}", name=f"ctxT{g}")
                for g in range(2)
            ]

            # ---- phase 0: HAM warmup — ~6µs of dep-free back-to-back
            # matmuls so the PE clock gate opens (1.2 -> 2.4 GHz) before the
            # real work; garbage values land in a scratch PSUM bank that is
            # never read.  A dummy exp on ScalarE pulls the ~2.7µs ACT
            # table load off the critical path too. ----
            with tc.tile_pool(name="pp", bufs=4, space="PSUM") as pp:
                warm_sb = const.tile([128, 512], BF16, tag="warm")
                nc.vector.memset(warm_sb, 0.0)
                dume_sb = const.tile([1, 16], BF16, tag="dume")
                nc.scalar.activation(
                    out=dume_sb, in_=warm_sb[0:1, 0:16], func=EXP, scale=SCALE
                )
                warm_ps = pp.tile([128, 512], F32, tag="pp")
                for _ in range(12):
                    nc.tensor.matmul(
                        out=warm_ps, lhsT=warm_sb[:, 0:128], rhs=warm_sb[:, :],
                        start=True, stop=True,
                    )

                # ---- phase 1: qT/kT [feature, s] = w_qkv^T @ x; bias folded
                #      into the eviction (per-partition, features-major),
                #      evictions alternating ScalarE/VectorE ----
                for t in range(4):  # feature tiles: q0,q1,k0,k1
                    for nb in range(4):  # s blocks of 512
                        ps = pp.tile([128, 512], F32, tag="pp")
                        for ks in range(2):
                            nc.tensor.matmul(
                                out=ps,
                                lhsT=wqk_sb[:, ks * 512 + t * 128 : ks * 512 + t * 128 + 128],
                                rhs=xt_sb[:, ks * S + nb * 512 : ks * S + nb * 512 + 512],
                                start=(ks == 0), stop=(ks == 1),
                            )
                        dst = (qT_sb if t < 2 else kT_sb)[
                            :, (t % 2) * S + nb * 512 : (t % 2) * S + nb * 512 + 512
                        ]
                        if (t * 4 + nb) % 2 == 0:
                            nc.scalar.activation(
                                out=dst, in_=ps, func=IDN,
                                bias=bqkc_sb[:, t : t + 1], scale=1.0,
                            )
                        else:
                            nc.vector.tensor_scalar_add(
                                out=dst, in0=ps, scalar1=bqkc_sb[:, t : t + 1]
                            )

                # ---- phase 2: v (natural layout, 32-wide head slots),
                #      evictions alternating ScalarE/VectorE ----
                for st in range(16):
                    ps = pp.tile([128, 256], F32, tag="ppv")
                    for ks in range(2):
                        nc.tensor.matmul(
                            out=ps,
                            lhsT=xt_sb[:, ks * S + st * 128 : ks * S + st * 128 + 128],
                            rhs=wv_sb[:, ks * 256 : ks * 256 + 256],
                            start=(ks == 0), stop=False,
                        )
                    nc.tensor.matmul(
                        out=ps,
                        lhsT=ones1_sb[0:1, 0:128],
                        rhs=bv_sb[0:1, 0:256],
                        start=False, stop=True,
                    )
                    dst = v_sb[:, st * 256 : st * 256 + 256]
                    if st % 2 == 0:
                        nc.scalar.copy(out=dst, in_=ps)
                    else:
                        nc.vector.tensor_copy(out=dst, in_=ps)

            # ---- phase 3: attention, q-blocks of 512 ----
            hidx = 0  # global exp-tile counter for the engine split

            def evict_qb(qb, ctx_ps, rs_ps):
                # engine copies out of PSUM (frees the accumulator banks),
                # then ship the 8 rowsum rows to DRAM and gather them back
                # as one [128, 32] tile
                stg = []
                for g in range(2):
                    sg = nrm.tile([128, 512], F32, tag=f"stg{g}", name=f"stg_{qb}_{g}")
                    nc.scalar.copy(out=sg, in_=ctx_ps[g])
                    stg.append(sg)
                rss = nrm.tile([128, 512], F32, tag="rss", name=f"rss_{qb}")
                nc.vector.tensor_copy(out=rss, in_=rs_ps)
                for g in range(2):
                    for hh in range(4):
                        r = qb * 8 + g * 4 + hh
                        nc.sync.dma_start(
                            out=rscr[r : r + 1, :],
                            in_=rss[hh * 32 + g : hh * 32 + g + 1, :],
                        )
                rsg = nrm.tile([128, 32], F32, tag="rsg", name=f"rsg_{qb}")
                nc.sync.dma_start(
                    out=rsg,
                    in_=rscr[qb * 8 : qb * 8 + 8, :].rearrange("r (c k) -> (r c) k", k=32),
                )
                return stg, rsg

            def norm_a(qb, stg, rsg):
                # reciprocal + scatter + partition-broadcast roundtrip
                rsgi = nrm.tile([128, 32], F32, tag="rsgi", name=f"rsgi_{qb}")
                nc.vector.reciprocal(out=rsgi, in_=rsg)
                nc.sync.dma_start(
                    out=rscr2[qb * 8 : qb * 8 + 8, :].rearrange("r (c k) -> (r c) k", k=32),
                    in_=rsgi,
                )
                rcb = []
                for g in range(2):
                    rc = nrm.tile([128, 512], F32, tag=f"rcb{g}", name=f"rcb_{qb}_{g}")
                    for hh in range(4):
                        r = qb * 8 + g * 4 + hh
                        nc.sync.dma_start(
                            out=rc[hh * 32 : hh * 32 + 32, :],
                            in_=rscr2[r : r + 1, :].to_broadcast((32, 512)),
                        )
                    rcb.append(rc)
                return rcb

            def norm_b(qb, stg, rcb):
                for g in range(2):
                    nc.vector.tensor_mul(
                        out=ctxT_sb[g][:, qb * 512 : qb * 512 + 512],
                        in0=stg[g], in1=rcb[g],
                    )

            with (
                tc.tile_pool(name="scp", bufs=2, space="PSUM") as scp,
                tc.tile_pool(name="cxp", bufs=1, space="PSUM") as cxp,
                tc.tile_pool(name="wkp", bufs=1, space="PSUM") as wkp,
            ):
                # warm-keeper: a garbage PSUM bank that dep-free filler
                # matmuls write into.  The fillers soak up every PE idle
                # sliver so the HAM activity monitor never re-throttles the
                # PE clock from 2.4 back to 1.2 GHz (observed: one MID
                # window fires ~20µs in and the whole attention loop runs
                # 2x slow without this).
                wk_ps = wkp.tile([128, 512], F32, tag="wk")

                def filler(n=128):
                    nc.tensor.matmul(
                        out=wk_ps[0:1, 0:n],
                        lhsT=onec_sb[:, 0:1],
                        rhs=warm_sb[:, 0:n],
                        start=True, stop=True, skip_group_check=True,
                    )

                pending_norm = None  # (qb, stg, rsg) awaiting recip+mul
                for qb in range(4):
                    ctx_ps = [
                        cxp.tile([128, 512], F32, tag=f"ctx{g}", name=f"ctx_{qb}_{g}")
                        for g in range(2)
                    ]
                    rs_ps = cxp.tile([128, 512], F32, tag="rs", name=f"rs_{qb}")

                    def emit_ctx(g, kt, eT):
                        # ctx + rowsum accumulation for (g, kt) — emitted one
                        # iteration late so these PE matmuls fill the window
                        # while ACT/DVE run the *next* exp.  4 heads packed
                        # via 4x column tiling; rowsums are M=2 matmuls
                        # against the per-group selector columns, head
                        # (g,hh) landing in row hh*32+g of the shared bank.
                        for hh in range(4):
                            nc.tensor.matmul(
                                out=ctx_ps[g][hh * 32 : hh * 32 + 32, :],
                                lhsT=v_sb[:, kt * 256 + (g * 4 + hh) * 32 : kt * 256 + (g * 4 + hh) * 32 + 32],
                                rhs=eT[:, hh * 512 : hh * 512 + 512],
                                start=(kt == 0), stop=(kt == 15),
                                tile_position=(0, hh * 32), skip_group_check=True,
                            )
                        for hh in range(4):
                            nc.tensor.matmul(
                                out=rs_ps[hh * 32 : hh * 32 + 2, :],
                                lhsT=onesel_sb[:, 2 * g : 2 * g + 2],
                                rhs=eT[:, hh * 512 : hh * 512 + 512],
                                start=(kt == 0 and g == 0), stop=(kt == 15 and g == 1),
                                tile_position=(0, hh * 32), skip_group_check=True,
                            )

                    pending = None
                    for kt in range(16):
                        for g in range(2):  # head groups of 4
                            # normalize for the previous q-block, emitted here
                            # so its DMA roundtrips overlap exp work instead
                            # of blocking the VectorE queue
                            if pending_norm is not None and g == 0:
                                pqb, pstg, prsg = pending_norm
                                if kt == 1:
                                    pending_norm = (pqb, pstg, norm_a(pqb, pstg, prsg))
                                elif kt == 3:
                                    norm_b(pqb, pstg, pending_norm[2])
                                    pending_norm = None
                            eT = etp.tile([128, 2048], BF16, tag="eT")
                            # two half-groups in separate PSUM tiles: the
                            # half-B exp's WAR doesn't block half-A scores,
                            # so the next scores always overlap the running
                            # exp and the exp engines never wait on the PE
                            for half in range(2):
                                sc = scp.tile([128, 1024], F32, tag="sc",
                                              name=f"sc_{qb}_{kt}_{g}_{half}")
                                for i in (2 * half, 2 * half + 1):
                                    nc.tensor.matmul(
                                        out=sc[:, (i % 2) * 512 : (i % 2) * 512 + 512],
                                        lhsT=kT_sb[32 * i : 32 * i + 32,
                                                   g * S + kt * 128 : g * S + kt * 128 + 128],
                                        rhs=qT_sb[32 * i : 32 * i + 32,
                                                  g * S + qb * 512 : g * S + qb * 512 + 512],
                                        start=True, stop=True,
                                        tile_position=(32 * i, 0),
                                    )
                                edst = eT[:, half * 1024 : half * 1024 + 1024]
                                if (hidx % 16) in DVE_SLOTS:
                                    nc.vector.tensor_scalar(
                                        out=edst.bitcast(mybir.dt.int16),
                                        in0=sc, scalar1=A16, scalar2=B16,
                                        op0=MUL, op1=ADD,
                                    )
                                else:
                                    nc.scalar.activation(
                                        out=edst, in_=sc, func=EXP, scale=SCALE,
                                    )
                                hidx += 1
                            if pending is not None:
                                emit_ctx(*pending)
                            pending = (g, kt, eT)
                            # keep the PE stream gapless; extra budget while
                            # the exp pipeline fills at the very start
                            if qb == 0 and kt == 0:
                                for _ in range(5):
                                    filler(512)
                            else:
                                filler(128)
                                filler(128)
                    emit_ctx(*pending)
                    stg, rsg = evict_qb(qb, ctx_ps, rs_ps)
                    pending_norm = (qb, stg, rsg)

                # tail: normalize the last q-block
                pqb, pstg, prsg = pending_norm
                rcb = norm_a(pqb, pstg, prsg)
                norm_b(pqb, pstg, rcb)

            # ---- phase 4: out = ctxT^T @ w_out + b_out (K=1 ones-row
            #      matmul adds the bias) ----
            with (
                tc.tile_pool(name="op", bufs=4, space="PSUM") as op,
                tc.tile_pool(name="ev", bufs=4) as ev,
            ):
                for st in range(16):
                    ps = op.tile([128, 256], F32, tag="op")
                    nc.tensor.matmul(
                        out=ps, lhsT=ones1_sb[0:1, 0:128], rhs=bo_sb[0:1, :],
                        start=True, stop=False,
                    )
                    for g in range(2):
                        nc.tensor.matmul(
                            out=ps,
                            lhsT=ctxT_sb[g][:, st * 128 : st * 128 + 128],
                            rhs=wo_sb[:, g * 256 : g * 256 + 256],
                            start=False, stop=(g == 1),
                        )
                    ot = ev.tile([128, 256], F32, tag="ot")
                    if st % 2 == 0:
                        nc.scalar.copy(out=ot, in_=ps)
                    else:
                        nc.vector.tensor_copy(out=ot, in_=ps)
                    nc.sync.dma_start(
                        out=out[st * 128 : st * 128 + 128, :], in_=ot
                    )
    if legalize:
        _legalize_sync_waits(nc)
    return nc


_NC_CACHE = None


def _get_nc():
    global _NC_CACHE
    if _NC_CACHE is None:
        _NC_CACHE = _build_nc()
    return _NC_CACHE


def _ks_layout(a, nk, cols):
    """[nk*128, cols] -> [128, nk*cols] with [p, k*cols+c] = a[k*128+p, c]."""
    return np.ascontiguousarray(
        a.reshape(nk, 128, cols).transpose(1, 0, 2).reshape(128, nk * cols)
    )


def _prep_in_maps(x, w_qkv, b_qkv, w_out, b_out):
    x = np.asarray(x, dtype=np.float32)
    w_qkv = np.asarray(w_qkv, dtype=np.float32)
    b_qkv = np.asarray(b_qkv, dtype=np.float32)
    w_out = np.asarray(w_out, dtype=np.float32)
    b_out = np.asarray(b_out, dtype=np.float32)

    # shared (per-core identical) weight layouts
    wqk_l = _ks_layout(w_qkv[:, : 2 * H], 2, 512).astype(NPBF16)
    # v weights in natural head order (32-wide slots)
    wv_l = _ks_layout(w_qkv[:, 2 * H :], 2, 256).astype(NPBF16)
    # out projection: ctxT is packed [(head h)*32 + d] so w_out needs no
    # permutation, only the K-split layout
    wo_l = _ks_layout(w_out, 2, 256).astype(NPBF16)

    shared = {
        "wqk": wqk_l,
        "wv": wv_l,
        "bv": b_qkv[2 * H :].reshape(1, H).astype(NPBF16),
        "wo": wo_l,
        "bo": b_out.reshape(1, H).astype(NPBF16),
        "bqkc": np.ascontiguousarray(
            b_qkv[: 2 * H].astype(np.float32).reshape(4, 128).T
        ),
        "ones": np.ones((1, 512), NPBF16),
    }
    in_maps = []
    for b in range(B):
        xt = _ks_layout(np.ascontiguousarray(x[b].T), 2, S).astype(NPBF16)
        in_maps.append({"xt": xt, **shared})
    return in_maps


def kernel(x, w_qkv, b_qkv, w_out, b_out):
    in_maps = _prep_in_maps(x, w_qkv, b_qkv, w_out, b_out)
    nc = _get_nc()
    res = run_bass_kernel_spmd(nc, in_maps, list(range(N_CORES)), **TRACE_OPTS)
    global LAST_RESULT
    LAST_RESULT = res
    return np.stack([res.results[b]["out"] for b in range(B)], axis=0)


# revision 13
# speedup vs baseline: 1.0892x; 1.0061x over previous
"""Multi-head self-attention (B=8, S=2048, H=256, NH=8, HD=32) on 8 TRN2 cores.

Strategy: data-parallel over batch — each core computes full MHA for one
batch element; no collectives.

Per-core dataflow (all matmuls bf16 in / fp32 PSUM accum):
  - host ships x^T (features on partitions) so no on-device transpose
  - qkT:  q^T,k^T [feat, s] = w_qkv^T @ x — feature-major so each head's
    32 q/k features land on one 32-partition strip; bias folded into the
    eviction (split between ScalarE and VectorE, both idle here)
  - scores^T per (head, key-tile): 4 heads computed concurrently via
    4x row-tiled PE (tile_position=(32i,0), K=32)
  - softmax exp is the kernel bottleneck (NH*S*S = 33.5M elements/core,
    and exp natively runs only on ScalarE at 1 col/cycle).  The exp work
    is therefore SPLIT between two engines:
      * ScalarE share: ACTIVATE(Exp, scale=1/sqrt(HD)) from PSUM
      * VectorE share: one TENSOR_SCALAR computing the Schraudolph bit
        trick — i16 = trunc(score * (128*log2(e)/sqrt(HD)) + (127*128-C))
        written through an int16 bitcast of the bf16 eT tile.  The int16
        bit pattern IS bf16(exp(score/sqrt(HD))) up to ~2% sawtooth error
        which largely cancels under softmax renormalization (validated:
        global rel err 0.008 vs 0.005 for exact exp).
    The 16-slot assignment pattern interleaves the engines ~9:7 to
    balance ScalarE@1.2GHz against VectorE@0.96GHz + its other work.
  - softmax max-subtraction is skipped (scores are O(1), fp32 PSUM)
  - ctx^T accumulated over key tiles with 4x column-tiled PE
    (tile_position=(0,32h)): all 4 heads of a group land fully packed in
    ONE [128,512] PSUM bank; rowsums likewise accumulate as separate
    M=1 column-tiled matmuls (lhsT = ones column) into a second bank at
    partitions {0,32,64,96}
  - normalization off the critical path: ctx/rowsum banks evicted by
    single full-tile engine copies into [128,512] fp32 staging (frees
    the accumulators), rowsum rows shipped via DRAM into one [128,32]
    tile, one VectorE reciprocal, scattered back and partition-broadcast
    via DRAM, then 2 full-tile [128,512] multiplies write the normalized
    bf16 ctxT.  The VectorE pieces are emitted a few iterations into the
    NEXT q-block so the DMA round-trip never head-of-line-blocks the
    VectorE exp stream.
  - out = ctxT^T @ w_out + b_out: ctxT is fully packed (2 tiles of
    4 heads x 32 rows), w_out needs no permutation, bias comes from a
    K=1 ones-row matmul, and the result goes PSUM -> SBUF -> DRAM.
"""
import math

import numpy as np
import ml_dtypes

import bass_rust
import concourse.bass as bass
import concourse.mybir as mybir
import concourse.tile as tile
from concourse.vector_clock import ScopedClock
from concourse.bass_utils import run_bass_kernel_spmd

BF16 = mybir.dt.bfloat16
F32 = mybir.dt.float32
NPBF16 = ml_dtypes.bfloat16

B, S, H = 8, 2048, 256
NH, HD = 8, 32
SCALE = 1.0 / float(np.sqrt(HD))
N_CORES = 8

# Schraudolph-exp constants for the VectorE share: bf16 bit pattern of
# exp(SCALE*x) ~= trunc(x * A16 + B16) interpreted as int16.
A16 = SCALE * 128.0 / math.log(2.0)
B16 = 127.0 * 128.0 - 6.0

# Which of each 16 consecutive (qb,kt,g,half) exp tiles go to VectorE
# (7 of 16; the rest go to ScalarE).  Evenly interleaved.
DVE_SLOTS = frozenset(i for i in range(16) if (i * 7) % 16 < 7)

# Set by a test harness to collect HW timing: {"trace": bool, "trace_cores": [...]}
TRACE_OPTS = {}
LAST_RESULT = None

def _legalize_sync_waits(nc):
    """The walrus build here rejects >1 sync wait per instruction, but Tile
    freely emits 2-3 (and the exit drain up to ~27).  Move excess waits onto
    same-engine NoOp carriers inserted immediately before the offending
    instruction — identical semantics (the engine blocks on each wait in
    program order)."""
    n = 0
    for f in nc.m.functions:
        for bb in f.blocks:
            insts = bb.instructions  # live list
            i = 0
            while i < len(insts):
                inst = insts[i]
                si = inst.sync_info
                if si is not None and len(si.on_wait) > 1:
                    waits = list(si.on_wait)
                    carriers = []
                    for w in waits[:-1]:
                        carriers.append(
                            mybir.InstNoOp(
                                name=f"{inst.name}-w{n}",
                                sync_info=mybir.SyncInfo(on_wait=[w], on_update=[]),
                                bass_nofuse=True,
                                engine=inst.engine,
                            )
                        )
                        n += 1
                    inst.sync_info = bass_rust.SyncInfo(
                        on_wait=waits[-1:], on_update=list(si.on_update)
                    )
                    insts[i:i] = carriers
                    i += len(carriers)
                i += 1
    return n


def _build_nc(legalize=True):
    nc = bass.Bass()
    xt = nc.dram_tensor("xt", [128, 2 * S], BF16, kind="ExternalInput")
    wqk = nc.dram_tensor("wqk", [128, 2 * 512], BF16, kind="ExternalInput")
    bv = nc.dram_tensor("bv", [1, 256], BF16, kind="ExternalInput")
    wv = nc.dram_tensor("wv", [128, 2 * 256], BF16, kind="ExternalInput")
    wo = nc.dram_tensor("wo", [128, 2 * 256], BF16, kind="ExternalInput")
    bo = nc.dram_tensor("bo", [1, 256], BF16, kind="ExternalInput")
    bqkc = nc.dram_tensor("bqkc", [128, 4], F32, kind="ExternalInput")
    ones = nc.dram_tensor("ones", [1, 512], BF16, kind="ExternalInput")
    out = nc.dram_tensor("out", [S, H], F32, kind="ExternalOutput")
    # scratch for the rowsum gather / reciprocal-broadcast DMA roundtrips
    # (SBUF APs cannot have a zero partition step, DRAM APs can)
    rscr = nc.dram_tensor("rscr", [32, 512], F32)
    rscr2 = nc.dram_tensor("rscr2", [32, 512], F32)

    EXP = mybir.ActivationFunctionType.Exp
    IDN = mybir.ActivationFunctionType.Identity
    MUL = mybir.AluOpType.mult
    ADD = mybir.AluOpType.add

    with tile.TileContext(nc) as tc:
        with (
            tc.tile_pool(name="const", bufs=1) as const,
            tc.tile_pool(name="etp", bufs=4) as etp,
            tc.tile_pool(name="nrm", bufs=2) as nrm,
        ):
            xt_sb = const.tile([128, 2 * S], BF16, tag="xt")
            nc.sync.dma_start(out=xt_sb, in_=xt[:, :])
            wqk_sb = const.tile([128, 2 * 512], BF16, tag="wqk")
            nc.sync.dma_start(out=wqk_sb, in_=wqk[:, :])
            wv_sb = const.tile([128, 2 * 256], BF16, tag="wv")
            nc.sync.dma_start(out=wv_sb, in_=wv[:, :])
            wo_sb = const.tile([128, 2 * 256], BF16, tag="wo")
            nc.sync.dma_start(out=wo_sb, in_=wo[:, :])
            bv_sb = const.tile([1, 256], BF16, tag="bv")
            nc.sync.dma_start(out=bv_sb, in_=bv[:, :])
            bo_sb = const.tile([1, 256], BF16, tag="bo")
            nc.sync.dma_start(out=bo_sb, in_=bo[:, :])
            ones1_sb = const.tile([1, 128], BF16, tag="ones1")
            nc.sync.dma_start(out=ones1_sb, in_=ones[0:1, 0:128])
            bqkc_sb = const.tile([128, 4], F32, tag="bqkc")
            nc.sync.dma_start(out=bqkc_sb, in_=bqkc[:, :])
            # rowsum selector columns (K=128, M=2): group 0 contracts
            # against [1|0] (sum lands in row +0), group 1 against [0|1]
            # (row +1) — both groups share one PSUM rowsum bank
            onesel_sb = const.tile([128, 4], BF16, tag="onesel")
            nc.vector.memset(onesel_sb[:, 0:1], 1.0)
            nc.vector.memset(onesel_sb[:, 1:3], 0.0)
            nc.vector.memset(onesel_sb[:, 3:4], 1.0)
            # ones column for warm-keeper filler matmuls
            onec_sb = const.tile([128, 1], BF16, tag="onec")
            nc.vector.memset(onec_sb, 1.0)

            qT_sb = const.tile([128, 2 * S], BF16, tag="qT")
            kT_sb = const.tile([128, 2 * S], BF16, tag="kT")
            v_sb = const.tile([128, 16 * 256], BF16, tag="v")
            # fully-packed normalized ctx^T: tile g holds heads 4g..4g+3,
            # rows h*32..h*32+32 = head (4g+h) features, cols = q positions
            ctxT_sb = [
                const.tile([128, S], BF16, tag=f"ctxT{g}", name=f"ctxT{g}")
                for g in range(2)
            ]

            # ---- phase 0: HAM warmup — ~6µs of dep-free back-to-back
            # matmuls so the PE clock gate opens (1.2 -> 2.4 GHz) before the
            # real work; garbage values land in a scratch PSUM bank that is
            # never read.  A dummy exp on ScalarE pulls the ~2.7µs ACT
            # table load off the critical path too. ----
            with tc.tile_pool(name="pp", bufs=4, space="PSUM") as pp:
                warm_sb = const.tile([128, 512], BF16, tag="warm")
                nc.vector.memset(warm_sb, 0.0)
                dume_sb = const.tile([1, 16], BF16, tag="dume")
                nc.scalar.activation(
                    out=dume_sb, in_=warm_sb[0:1, 0:16], func=EXP, scale=SCALE
                )
                warm_ps = pp.tile([128, 512], F32, tag="pp")
                for _ in range(12):
                    nc.tensor.matmul(
                        out=warm_ps, lhsT=warm_sb[:, 0:128], rhs=warm_sb[:, :],
                        start=True, stop=True,
                    )

                # ---- phase 1: qT/kT [feature, s] = w_qkv^T @ x; bias folded
                #      into the eviction (per-partition, features-major),
                #      evictions alternating ScalarE/VectorE ----
                for t in range(4):  # feature tiles: q0,q1,k0,k1
                    for nb in range(4):  # s blocks of 512
                        ps = pp.tile([128, 512], F32, tag="pp")
                        for ks in range(2):
                            nc.tensor.matmul(
                                out=ps,
                                lhsT=wqk_sb[:, ks * 512 + t * 128 : ks * 512 + t * 128 + 128],
                                rhs=xt_sb[:, ks * S + nb * 512 : ks * S + nb * 512 + 512],
                                start=(ks == 0), stop=(ks == 1),
                            )
                        dst = (qT_sb if t < 2 else kT_sb)[
                            :, (t % 2) * S + nb * 512 : (t % 2) * S + nb * 512 + 512
                        ]
                        if (t * 4 + nb) % 2 == 0:
                            nc.scalar.activation(
                                out=dst, in_=ps, func=IDN,
                                bias=bqkc_sb[:, t : t + 1], scale=1.0,
                            )
                        else:
                            nc.vector.tensor_scalar_add(
                                out=dst, in0=ps, scalar1=bqkc_sb[:, t : t + 1]
                            )

                # ---- phase 2: v (natural layout, 32-wide head slots),
                #      evictions alternating ScalarE/VectorE ----
                for st in range(16):
                    ps = pp.tile([128, 256], F32, tag="ppv")
                    for ks in range(2):
                        nc.tensor.matmul(
                            out=ps,
                            lhsT=xt_sb[:, ks * S + st * 128 : ks * S + st * 128 + 128],
                            rhs=wv_sb[:, ks * 256 : ks * 256 + 256],
                            start=(ks == 0), stop=False,
                        )
                    nc.tensor.matmul(
                        out=ps,
                        lhsT=ones1_sb[0:1, 0:128],
                        rhs=bv_sb[0:1, 0:256],
                        start=False, stop=True,
                    )
                    dst = v_sb[:, st * 256 : st * 256 + 256]
                    if st % 2 == 0:
                        nc.scalar.copy(out=dst, in_=ps)
                    else:
                        nc.vector.tensor_copy(out=dst, in_=ps)

            # ---- phase 3: attention, q-blocks of 512 ----
            hidx = 0  # global exp-tile counter for the engine split

            def evict_qb(qb, ctx_ps, rs_ps):
                # engine copies out of PSUM (frees the accumulator banks),
                # then ship the 8 rowsum rows to DRAM and gather them back
                # as one [128, 32] tile
                stg = []
                for g in range(2):
                    sg = nrm.tile([128, 512], F32, tag=f"stg{g}", name=f"stg_{qb}_{g}")
                    nc.scalar.copy(out=sg, in_=ctx_ps[g])
                    stg.append(sg)
                rss = nrm.tile([128, 512], F32, tag="rss", name=f"rss_{qb}")
                nc.vector.tensor_copy(out=rss, in_=rs_ps)
                for g in range(2):
                    for hh in range(4):
                        r = qb * 8 + g * 4 + hh
                        nc.sync.dma_start(
                            out=rscr[r : r + 1, :],
                            in_=rss[hh * 32 + g : hh * 32 + g + 1, :],
                        )
                rsg = nrm.tile([128, 32], F32, tag="rsg", name=f"rsg_{qb}")
                nc.sync.dma_start(
                    out=rsg,
                    in_=rscr[qb * 8 : qb * 8 + 8, :].rearrange("r (c k) -> (r c) k", k=32),
                )
                return stg, rsg

            def norm_a(qb, stg, rsg):
                # reciprocal + scatter + partition-broadcast roundtrip
                rsgi = nrm.tile([128, 32], F32, tag="rsgi", name=f"rsgi_{qb}")
                nc.vector.reciprocal(out=rsgi, in_=rsg)
                nc.sync.dma_start(
                    out=rscr2[qb * 8 : qb * 8 + 8, :].rearrange("r (c k) -> (r c) k", k=32),
                    in_=rsgi,
                )
                rcb = []
                for g in range(2):
                    rc = nrm.tile([128, 512], F32, tag=f"rcb{g}", name=f"rcb_{qb}_{g}")
                    for hh in range(4):
                        r = qb * 8 + g * 4 + hh
                        nc.sync.dma_start(
                            out=rc[hh * 32 : hh * 32 + 32, :],
                            in_=rscr2[r : r + 1, :].to_broadcast((32, 512)),
                        )
                    rcb.append(rc)
                return rcb

            def norm_b(qb, stg, rcb):
                for g in range(2):
                    nc.vector.tensor_mul(
                        out=ctxT_sb[g][:, qb * 512 : qb * 512 + 512],
                        in0=stg[g], in1=rcb[g],
                    )

            with (
                tc.tile_pool(name="scp", bufs=2, space="PSUM") as scp,
                tc.tile_pool(name="cxp", bufs=1, space="PSUM") as cxp,
                tc.tile_pool(name="wkp", bufs=1, space="PSUM") as wkp,
            ):
                # warm-keeper: a garbage PSUM bank that dep-free filler
                # matmuls write into.  The fillers soak up every PE idle
                # sliver so the HAM activity monitor never re-throttles the
                # PE clock from 2.4 back to 1.2 GHz (observed: one MID
                # window fires ~20µs in and the whole attention loop runs
                # 2x slow without this).
                wk_ps = wkp.tile([128, 512], F32, tag="wk")

                def filler(n=128):
                    nc.tensor.matmul(
                        out=wk_ps[0:1, 0:n],
                        lhsT=onec_sb[:, 0:1],
                        rhs=warm_sb[:, 0:n],
                        start=True, stop=True, skip_group_check=True,
                    )

                pending_norm = None  # (qb, stg, rsg) awaiting recip+mul
                pending_evict = None  # (qb, ctx_ps, rs_ps) awaiting eviction
                for qb in range(4):
                    ctx_ps = [
                        cxp.tile([128, 512], F32, tag=f"ctx{g}", name=f"ctx_{qb}_{g}")
                        for g in range(2)
                    ]
                    rs_ps = cxp.tile([128, 512], F32, tag="rs", name=f"rs_{qb}")

                    def emit_ctx(g, kt, eT):
                        # ctx + rowsum accumulation for (g, kt) — emitted one
                        # iteration late so these PE matmuls fill the window
                        # while ACT/DVE run the *next* exp.  4 heads packed
                        # via 4x column tiling; rowsums are M=2 matmuls
                        # against the per-group selector columns, head
                        # (g,hh) landing in row hh*32+g of the shared bank.
                        for hh in range(4):
                            nc.tensor.matmul(
                                out=ctx_ps[g][hh * 32 : hh * 32 + 32, :],
                                lhsT=v_sb[:, kt * 256 + (g * 4 + hh) * 32 : kt * 256 + (g * 4 + hh) * 32 + 32],
                                rhs=eT[:, hh * 512 : hh * 512 + 512],
                                start=(kt == 0), stop=(kt == 15),
                                tile_position=(0, hh * 32), skip_group_check=True,
                            )
                        for hh in range(4):
                            nc.tensor.matmul(
                                out=rs_ps[hh * 32 : hh * 32 + 2, :],
                                lhsT=onesel_sb[:, 2 * g : 2 * g + 2],
                                rhs=eT[:, hh * 512 : hh * 512 + 512],
                                start=(kt == 0 and g == 0), stop=(kt == 15 and g == 1),
                                tile_position=(0, hh * 32), skip_group_check=True,
                            )

                    pending = None
                    for kt in range(16):
                        for g in range(2):  # head groups of 4
                            # eviction + normalize for the previous q-block
                            # are emitted a few blocks into THIS one: the
                            # PE's final ctx matmuls are certainly done by
                            # then (no engine-queue stall) and the DMA
                            # roundtrips overlap exp work instead of
                            # head-of-line-blocking the exp streams
                            if pending_evict is not None and kt == 0 and g == 1:
                                pqb, pctx, prs = pending_evict
                                pending_norm = (pqb,) + evict_qb(pqb, pctx, prs)
                                pending_evict = None
                            if pending_norm is not None and g == 0:
                                pqb, pstg, prsg = pending_norm
                                if kt == 2:
                                    pending_norm = (pqb, pstg, norm_a(pqb, pstg, prsg))
                                elif kt == 4:
                                    norm_b(pqb, pstg, pending_norm[2])
                                    pending_norm = None
                            eT = etp.tile([128, 2048], BF16, tag="eT")
                            # two half-groups in separate PSUM tiles: the
                            # half-B exp's WAR doesn't block half-A scores,
                            # so the next scores always overlap the running
                            # exp and the exp engines never wait on the PE
                            for half in range(2):
                                sc = scp.tile([128, 1024], F32, tag="sc",
                                              name=f"sc_{qb}_{kt}_{g}_{half}")
                                for i in (2 * half, 2 * half + 1):
                                    nc.tensor.matmul(
                                        out=sc[:, (i % 2) * 512 : (i % 2) * 512 + 512],
                                        lhsT=kT_sb[32 * i : 32 * i + 32,
                                                   g * S + kt * 128 : g * S + kt * 128 + 128],
                                        rhs=qT_sb[32 * i : 32 * i + 32,
                                                  g * S + qb * 512 : g * S + qb * 512 + 512],
                                        start=True, stop=True,
                                        tile_position=(32 * i, 0),
                                    )
                                edst = eT[:, half * 1024 : half * 1024 + 1024]
                                if (hidx % 16) in DVE_SLOTS:
                                    nc.vector.tensor_scalar(
                                        out=edst.bitcast(mybir.dt.int16),
                                        in0=sc, scalar1=A16, scalar2=B16,
                                        op0=MUL, op1=ADD,
                                    )
                                else:
                                    nc.scalar.activation(
                                        out=edst, in_=sc, func=EXP, scale=SCALE,
                                    )
                                hidx += 1
                            if pending is not None:
                                emit_ctx(*pending)
                            pending = (g, kt, eT)
                            # keep the PE stream gapless; extra budget while
                            # the exp pipeline fills at the very start and
                            # around q-block boundaries
                            if qb == 0 and kt == 0:
                                for _ in range(5):
                                    filler(512)
                            elif kt in (15, 0, 1):
                                filler(512)
                                filler(256)
                            else:
                                filler(192)
                                filler(192)
                    emit_ctx(*pending)
                    pending_evict = (qb, ctx_ps, rs_ps)

                # tail: evict + normalize the last q-block
                pqb, pctx, prs = pending_evict
                pstg, prsg = evict_qb(pqb, pctx, prs)
                rcb = norm_a(pqb, pstg, prsg)
                norm_b(pqb, pstg, rcb)

            # ---- phase 4: out = ctxT^T @ w_out + b_out (K=1 ones-row
            #      matmul adds the bias) ----
            with (
                tc.tile_pool(name="op", bufs=4, space="PSUM") as op,
                tc.tile_pool(name="ev", bufs=4) as ev,
            ):
                for st in range(16):
                    ps = op.tile([128, 256], F32, tag="op")
                    nc.tensor.matmul(
                        out=ps, lhsT=ones1_sb[0:1, 0:128], rhs=bo_sb[0:1, :],
                        start=True, stop=False,
                    )
                    for g in range(2):
                        nc.tensor.matmul(
                            out=ps,
                            lhsT=ctxT_sb[g][:, st * 128 : st * 128 + 128],
                            rhs=wo_sb[:, g * 256 : g * 256 + 256],
                            start=False, stop=(g == 1),
                        )
                    ot = ev.tile([128, 256], F32, tag="ot")
                    if st % 2 == 0:
                        nc.scalar.copy(out=ot, in_=ps)
                    else:
                        nc.vector.tensor_copy(out=ot, in_=ps)
                    nc.sync.dma_start(
                        out=out[st * 128 : st * 128 + 128, :], in_=ot
                    )
    if legalize:
        _legalize_sync_waits(nc)
    return nc


_NC_CACHE = None


def _get_nc():
    global _NC_CACHE
    if _NC_CACHE is None:
        _NC_CACHE = _build_nc()
    return _NC_CACHE


def _ks_layout(a, nk, cols):
    """[nk*128, cols] -> [128, nk*cols] with [p, k*cols+c] = a[k*128+p, c]."""
    return np.ascontiguousarray(
        a.reshape(nk, 128, cols).transpose(1, 0, 2).reshape(128, nk * cols)
    )


def _prep_in_maps(x, w_qkv, b_qkv, w_out, b_out):
    x = np.asarray(x, dtype=np.float32)
    w_qkv = np.asarray(w_qkv, dtype=np.float32)
    b_qkv = np.asarray(b_qkv, dtype=np.float32)
    w_out = np.asarray(w_out, dtype=np.float32)
    b_out = np.asarray(b_out, dtype=np.float32)

    # shared (per-core identical) weight layouts
    wqk_l = _ks_layout(w_qkv[:, : 2 * H], 2, 512).astype(NPBF16)
    # v weights in natural head order (32-wide slots)
    wv_l = _ks_layout(w_qkv[:, 2 * H :], 2, 256).astype(NPBF16)
    # out projection: ctxT is packed [(head h)*32 + d] so w_out needs no
    # permutation, only the K-split layout
    wo_l = _ks_layout(w_out, 2, 256).astype(NPBF16)

    shared = {
        "wqk": wqk_l,
        "wv": wv_l,
        "bv": b_qkv[2 * H :].reshape(1, H).astype(NPBF16),
        "wo": wo_l,
        "bo": b_out.reshape(1, H).astype(NPBF16),
        "bqkc": np.ascontiguousarray(
            b_qkv[: 2 * H].astype(np.float32).reshape(4, 128).T
        ),
        "ones": np.ones((1, 512), NPBF16),
    }
    in_maps = []
    for b in range(B):
        xt = _ks_layout(np.ascontiguousarray(x[b].T), 2, S).astype(NPBF16)
        in_maps.append({"xt": xt, **shared})
    return in_maps


def kernel(x, w_qkv, b_qkv, w_out, b_out):
    in_maps = _prep_in_maps(x, w_qkv, b_qkv, w_out, b_out)
    nc = _get_nc()
    res = run_bass_kernel_spmd(nc, in_maps, list(range(N_CORES)), **TRACE_OPTS)
    global LAST_RESULT
    LAST_RESULT = res
    return np.stack([res.results[b]["out"] for b in range(B)], axis=0)


# revision 15
# speedup vs baseline: 1.2005x; 1.1022x over previous
"""Multi-head self-attention (B=8, S=2048, H=256, NH=8, HD=32) on 8 TRN2 cores.

Strategy: data-parallel over batch — each core computes full MHA for one
batch element; no collectives.

Per-core dataflow (all matmuls bf16 in / fp32 PSUM accum):
  - host ships x^T (features on partitions) so no on-device transpose
  - qkT:  q^T,k^T [feat, s] = w_qkv^T @ x — feature-major so each head's
    32 q/k features land on one 32-partition strip; bias folded into the
    eviction (split between ScalarE and VectorE, both idle here)
  - scores^T per (head, key-tile): 4 heads computed concurrently via
    4x row-tiled PE (tile_position=(32i,0), K=32)
  - softmax exp is the kernel bottleneck (NH*S*S = 33.5M elements/core,
    and exp natively runs only on ScalarE at 1 col/cycle).  The exp work
    is therefore SPLIT between two engines:
      * ScalarE share: ACTIVATE(Exp, scale=1/sqrt(HD)) from PSUM
      * VectorE share: one TENSOR_SCALAR computing the Schraudolph bit
        trick — i16 = trunc(score * (128*log2(e)/sqrt(HD)) + (127*128-C))
        written through an int16 bitcast of the bf16 eT tile.  The int16
        bit pattern IS bf16(exp(score/sqrt(HD))) up to ~2% sawtooth error
        which largely cancels under softmax renormalization (validated:
        global rel err 0.008 vs 0.005 for exact exp).
    The 16-slot assignment pattern interleaves the engines ~9:7 to
    balance ScalarE@1.2GHz against VectorE@0.96GHz + its other work.
  - softmax max-subtraction is skipped (scores are O(1), fp32 PSUM)
  - ctx^T accumulated over key tiles with 4x column-tiled PE
    (tile_position=(0,32h)): all 4 heads of a group land fully packed in
    ONE [128,512] PSUM bank; rowsums likewise accumulate as separate
    M=1 column-tiled matmuls (lhsT = ones column) into a second bank at
    partitions {0,32,64,96}
  - normalization off the critical path: ctx/rowsum banks evicted by
    single full-tile engine copies into [128,512] fp32 staging (frees
    the accumulators), rowsum rows shipped via DRAM into one [128,32]
    tile, one VectorE reciprocal, scattered back and partition-broadcast
    via DRAM, then 2 full-tile [128,512] multiplies write the normalized
    bf16 ctxT.  The VectorE pieces are emitted a few iterations into the
    NEXT q-block so the DMA round-trip never head-of-line-blocks the
    VectorE exp stream.
  - out = ctxT^T @ w_out + b_out: ctxT is fully packed (2 tiles of
    4 heads x 32 rows), w_out needs no permutation, bias comes from a
    K=1 ones-row matmul, and the result goes PSUM -> SBUF -> DRAM.
"""
import math

import numpy as np
import ml_dtypes

import bass_rust
import concourse.bass as bass
import concourse.mybir as mybir
import concourse.tile as tile
from concourse.vector_clock import ScopedClock
from concourse.bass_utils import run_bass_kernel_spmd

BF16 = mybir.dt.bfloat16
F32 = mybir.dt.float32
NPBF16 = ml_dtypes.bfloat16

B, S, H = 8, 2048, 256
NH, HD = 8, 32
SCALE = 1.0 / float(np.sqrt(HD))
N_CORES = 8

# Schraudolph-exp constants for the VectorE share: bf16 bit pattern of
# exp(SCALE*x) ~= trunc(x * A16 + B16) interpreted as int16.
A16 = SCALE * 128.0 / math.log(2.0)
B16 = 127.0 * 128.0 - 6.0

# Which of each 16 consecutive (qb,kt,g,half) exp tiles go to VectorE
# (7 of 16; the rest go to ScalarE).  Evenly interleaved.
DVE_SLOTS = frozenset(i for i in range(16) if (i * 7) % 16 < 7)

# Set by a test harness to collect HW timing: {"trace": bool, "trace_cores": [...]}
TRACE_OPTS = {}
LAST_RESULT = None

def _legalize_sync_waits(nc):
    """The walrus build here rejects >1 sync wait per instruction, but Tile
    freely emits 2-3 (and the exit drain up to ~27).  Move excess waits onto
    same-engine NoOp carriers inserted immediately before the offending
    instruction — identical semantics (the engine blocks on each wait in
    program order)."""
    n = 0
    for f in nc.m.functions:
        for bb in f.blocks:
            insts = bb.instructions  # live list
            i = 0
            while i < len(insts):
                inst = insts[i]
                si = inst.sync_info
                if si is not None and len(si.on_wait) > 1:
                    waits = list(si.on_wait)
                    carriers = []
                    for w in waits[:-1]:
                        carriers.append(
                            mybir.InstNoOp(
                                name=f"{inst.name}-w{n}",
                                sync_info=mybir.SyncInfo(on_wait=[w], on_update=[]),
                                bass_nofuse=True,
                                engine=inst.engine,
                            )
                        )
                        n += 1
                    inst.sync_info = bass_rust.SyncInfo(
                        on_wait=waits[-1:], on_update=list(si.on_update)
                    )
                    insts[i:i] = carriers
                    i += len(carriers)
                i += 1
    return n


def _build_nc(legalize=True):
    nc = bass.Bass()
    xt = nc.dram_tensor("xt", [128, 2 * S], BF16, kind="ExternalInput")
    wqk = nc.dram_tensor("wqk", [128, 2 * 512], BF16, kind="ExternalInput")
    bv = nc.dram_tensor("bv", [1, 256], BF16, kind="ExternalInput")
    wv = nc.dram_tensor("wv", [128, 2 * 256], BF16, kind="ExternalInput")
    wo = nc.dram_tensor("wo", [128, 2 * 256], BF16, kind="ExternalInput")
    bo = nc.dram_tensor("bo", [1, 256], BF16, kind="ExternalInput")
    bqkc = nc.dram_tensor("bqkc", [128, 4], F32, kind="ExternalInput")
    ones = nc.dram_tensor("ones", [1, 512], BF16, kind="ExternalInput")
    out = nc.dram_tensor("out", [S, H], F32, kind="ExternalOutput")
    # scratch for the rowsum gather / reciprocal-broadcast DMA roundtrips
    # (SBUF APs cannot have a zero partition step, DRAM APs can)
    rscr = nc.dram_tensor("rscr", [32, 512], F32)
    rscr2 = nc.dram_tensor("rscr2", [32, 512], F32)

    EXP = mybir.ActivationFunctionType.Exp
    IDN = mybir.ActivationFunctionType.Identity
    MUL = mybir.AluOpType.mult
    ADD = mybir.AluOpType.add

    with tile.TileContext(nc) as tc:
        with (
            tc.tile_pool(name="const", bufs=1) as const,
            tc.tile_pool(name="etp", bufs=4) as etp,
            tc.tile_pool(name="nrm", bufs=2) as nrm,
        ):
            xt_sb = const.tile([128, 2 * S], BF16, tag="xt")
            nc.sync.dma_start(out=xt_sb, in_=xt[:, :])
            wqk_sb = const.tile([128, 2 * 512], BF16, tag="wqk")
            nc.sync.dma_start(out=wqk_sb, in_=wqk[:, :])
            wv_sb = const.tile([128, 2 * 256], BF16, tag="wv")
            nc.sync.dma_start(out=wv_sb, in_=wv[:, :])
            wo_sb = const.tile([128, 2 * 256], BF16, tag="wo")
            nc.sync.dma_start(out=wo_sb, in_=wo[:, :])
            bv_sb = const.tile([1, 256], BF16, tag="bv")
            nc.sync.dma_start(out=bv_sb, in_=bv[:, :])
            bo_sb = const.tile([1, 256], BF16, tag="bo")
            nc.sync.dma_start(out=bo_sb, in_=bo[:, :])
            ones1_sb = const.tile([1, 128], BF16, tag="ones1")
            nc.sync.dma_start(out=ones1_sb, in_=ones[0:1, 0:128])
            bqkc_sb = const.tile([128, 4], F32, tag="bqkc")
            nc.sync.dma_start(out=bqkc_sb, in_=bqkc[:, :])
            # rowsum selector columns (K=128, M=2): group 0 contracts
            # against [1|0] (sum lands in row +0), group 1 against [0|1]
            # (row +1) — both groups share one PSUM rowsum bank
            onesel_sb = const.tile([128, 4], BF16, tag="onesel")
            nc.vector.memset(onesel_sb[:, 0:1], 1.0)
            nc.vector.memset(onesel_sb[:, 1:3], 0.0)
            nc.vector.memset(onesel_sb[:, 3:4], 1.0)
            # ones column for warm-keeper filler matmuls
            onec_sb = const.tile([128, 1], BF16, tag="onec")
            nc.vector.memset(onec_sb, 1.0)

            qT_sb = const.tile([128, 2 * S], BF16, tag="qT")
            kT_sb = const.tile([128, 2 * S], BF16, tag="kT")
            v_sb = const.tile([128, 16 * 256], BF16, tag="v")
            # fully-packed normalized ctx^T: tile g holds heads 4g..4g+3,
            # rows h*32..h*32+32 = head (4g+h) features, cols = q positions
            ctxT_sb = [
                const.tile([128, S], BF16, tag=f"ctxT{g}", name=f"ctxT{g}")
                for g in range(2)
            ]

            # ---- phase 0: HAM warmup — ~6µs of dep-free back-to-back
            # matmuls so the PE clock gate opens (1.2 -> 2.4 GHz) before the
            # real work; garbage values land in a scratch PSUM bank that is
            # never read.  A dummy exp on ScalarE pulls the ~2.7µs ACT
            # table load off the critical path too. ----
            with tc.tile_pool(name="pp", bufs=4, space="PSUM") as pp:
                warm_sb = const.tile([128, 512], BF16, tag="warm")
                nc.vector.memset(warm_sb, 0.0)
                dume_sb = const.tile([1, 16], BF16, tag="dume")
                nc.scalar.activation(
                    out=dume_sb, in_=warm_sb[0:1, 0:16], func=EXP, scale=SCALE
                )
                warm_ps = pp.tile([128, 512], F32, tag="pp")
                for _ in range(12):
                    nc.tensor.matmul(
                        out=warm_ps, lhsT=warm_sb[:, 0:128], rhs=warm_sb[:, :],
                        start=True, stop=True,
                    )

                # ---- phase 1: qT/kT [feature, s] = w_qkv^T @ x; bias folded
                #      into the eviction (per-partition, features-major),
                #      evictions alternating ScalarE/VectorE ----
                for t in range(4):  # feature tiles: q0,q1,k0,k1
                    for nb in range(4):  # s blocks of 512
                        ps = pp.tile([128, 512], F32, tag="pp")
                        for ks in range(2):
                            nc.tensor.matmul(
                                out=ps,
                                lhsT=wqk_sb[:, ks * 512 + t * 128 : ks * 512 + t * 128 + 128],
                                rhs=xt_sb[:, ks * S + nb * 512 : ks * S + nb * 512 + 512],
                                start=(ks == 0), stop=(ks == 1),
                            )
                        dst = (qT_sb if t < 2 else kT_sb)[
                            :, (t % 2) * S + nb * 512 : (t % 2) * S + nb * 512 + 512
                        ]
                        if (t * 4 + nb) % 2 == 0:
                            nc.scalar.activation(
                                out=dst, in_=ps, func=IDN,
                                bias=bqkc_sb[:, t : t + 1], scale=1.0,
                            )
                        else:
                            nc.vector.tensor_scalar_add(
                                out=dst, in0=ps, scalar1=bqkc_sb[:, t : t + 1]
                            )

                # ---- phase 2: v (natural layout, 32-wide head slots),
                #      evictions alternating ScalarE/VectorE ----
                for st in range(16):
                    ps = pp.tile([128, 256], F32, tag="ppv")
                    for ks in range(2):
                        nc.tensor.matmul(
                            out=ps,
                            lhsT=xt_sb[:, ks * S + st * 128 : ks * S + st * 128 + 128],
                            rhs=wv_sb[:, ks * 256 : ks * 256 + 256],
                            start=(ks == 0), stop=False,
                        )
                    nc.tensor.matmul(
                        out=ps,
                        lhsT=ones1_sb[0:1, 0:128],
                        rhs=bv_sb[0:1, 0:256],
                        start=False, stop=True,
                    )
                    dst = v_sb[:, st * 256 : st * 256 + 256]
                    if st % 2 == 0:
                        nc.scalar.copy(out=dst, in_=ps)
                    else:
                        nc.vector.tensor_copy(out=dst, in_=ps)

            # ---- phase 3: attention, q-blocks of 512 ----
            hidx = 0  # global exp-tile counter for the engine split

            def evict_qb(qb, ctx_ps, rs_ps):
                # engine copies out of PSUM (frees the accumulator banks),
                # then ship the 8 rowsum rows to DRAM and gather them back
                # as one [128, 32] tile
                stg = []
                for g in range(2):
                    sg = nrm.tile([128, 512], F32, tag=f"stg{g}", name=f"stg_{qb}_{g}")
                    nc.scalar.copy(out=sg, in_=ctx_ps[g])
                    stg.append(sg)
                rss = nrm.tile([128, 512], F32, tag="rss", name=f"rss_{qb}")
                nc.vector.tensor_copy(out=rss, in_=rs_ps)
                for g in range(2):
                    for hh in range(4):
                        r = qb * 8 + g * 4 + hh
                        nc.sync.dma_start(
                            out=rscr[r : r + 1, :],
                            in_=rss[hh * 32 + g : hh * 32 + g + 1, :],
                        )
                rsg = nrm.tile([128, 32], F32, tag="rsg", name=f"rsg_{qb}")
                nc.sync.dma_start(
                    out=rsg,
                    in_=rscr[qb * 8 : qb * 8 + 8, :].rearrange("r (c k) -> (r c) k", k=32),
                )
                return stg, rsg

            def norm_a(qb, stg, rsg):
                # reciprocal + scatter + partition-broadcast roundtrip
                rsgi = nrm.tile([128, 32], F32, tag="rsgi", name=f"rsgi_{qb}")
                nc.vector.reciprocal(out=rsgi, in_=rsg)
                nc.sync.dma_start(
                    out=rscr2[qb * 8 : qb * 8 + 8, :].rearrange("r (c k) -> (r c) k", k=32),
                    in_=rsgi,
                )
                rcb = []
                for g in range(2):
                    rc = nrm.tile([128, 512], F32, tag=f"rcb{g}", name=f"rcb_{qb}_{g}")
                    for hh in range(4):
                        r = qb * 8 + g * 4 + hh
                        nc.sync.dma_start(
                            out=rc[hh * 32 : hh * 32 + 32, :],
                            in_=rscr2[r : r + 1, :].to_broadcast((32, 512)),
                        )
                    rcb.append(rc)
                return rcb

            def norm_b(qb, stg, rcb):
                for g in range(2):
                    nc.vector.tensor_mul(
                        out=ctxT_sb[g][:, qb * 512 : qb * 512 + 512],
                        in0=stg[g], in1=rcb[g],
                    )

            with (
                tc.tile_pool(name="scp", bufs=4, space="PSUM") as scp,
                tc.tile_pool(name="cxp", bufs=1, space="PSUM") as cxp,
                tc.tile_pool(name="wkp", bufs=1, space="PSUM") as wkp,
            ):
                # warm-keeper: a garbage PSUM bank that dep-free filler
                # matmuls write into.  The fillers soak up every PE idle
                # sliver so the HAM activity monitor never re-throttles the
                # PE clock from 2.4 back to 1.2 GHz (observed: one MID
                # window fires ~20µs in and the whole attention loop runs
                # 2x slow without this).
                wk_ps = wkp.tile([128, 512], F32, tag="wk")

                def filler(n=128):
                    nc.tensor.matmul(
                        out=wk_ps[0:1, 0:n],
                        lhsT=onec_sb[:, 0:1],
                        rhs=warm_sb[:, 0:n],
                        start=True, stop=True, skip_group_check=True,
                    )

                pending_norm = None  # (qb, stg, rsg) awaiting recip+mul
                pending_evict = None  # (qb, ctx_ps, rs_ps) awaiting eviction
                for qb in range(4):
                    ctx_ps = [
                        cxp.tile([128, 512], F32, tag=f"ctx{g}", name=f"ctx_{qb}_{g}")
                        for g in range(2)
                    ]
                    rs_ps = cxp.tile([128, 512], F32, tag="rs", name=f"rs_{qb}")

                    def emit_ctx(g, kt, eT):
                        # ctx + rowsum accumulation for (g, kt) — emitted one
                        # iteration late so these PE matmuls fill the window
                        # while ACT/DVE run the *next* exp.  4 heads packed
                        # via 4x column tiling; rowsums are M=2 matmuls
                        # against the per-group selector columns, head
                        # (g,hh) landing in row hh*32+g of the shared bank.
                        for hh in range(4):
                            nc.tensor.matmul(
                                out=ctx_ps[g][hh * 32 : hh * 32 + 32, :],
                                lhsT=v_sb[:, kt * 256 + (g * 4 + hh) * 32 : kt * 256 + (g * 4 + hh) * 32 + 32],
                                rhs=eT[:, hh * 512 : hh * 512 + 512],
                                start=(kt == 0), stop=(kt == 15),
                                tile_position=(0, hh * 32), skip_group_check=True,
                            )
                        for hh in range(4):
                            nc.tensor.matmul(
                                out=rs_ps[hh * 32 : hh * 32 + 2, :],
                                lhsT=onesel_sb[:, 2 * g : 2 * g + 2],
                                rhs=eT[:, hh * 512 : hh * 512 + 512],
                                start=(kt == 0 and g == 0), stop=(kt == 15 and g == 1),
                                tile_position=(0, hh * 32), skip_group_check=True,
                            )

                    pending = None
                    for kt in range(16):
                        for g in range(2):  # head groups of 4
                            # eviction + normalize for the previous q-block
                            # are emitted a few blocks into THIS one: the
                            # PE's final ctx matmuls are certainly done by
                            # then (no engine-queue stall) and the DMA
                            # roundtrips overlap exp work instead of
                            # head-of-line-blocking the exp streams
                            if pending_evict is not None and kt == 0 and g == 1:
                                pqb, pctx, prs = pending_evict
                                pending_norm = (pqb,) + evict_qb(pqb, pctx, prs)
                                pending_evict = None
                            if pending_norm is not None and g == 0:
                                pqb, pstg, prsg = pending_norm
                                if kt == 2:
                                    pending_norm = (pqb, pstg, norm_a(pqb, pstg, prsg))
                                elif kt == 4:
                                    norm_b(pqb, pstg, pending_norm[2])
                                    pending_norm = None
                            eT = etp.tile([128, 2048], BF16, tag="eT")
                            # one PSUM quarter-tile per head: fine-grained
                            # WAR release (each exp frees its buffer in
                            # ~600ns) keeps a queued exp at BOTH engines,
                            # so neither the exp engines nor the PE sit in
                            # a cross-engine latency ping-pong
                            for q in range(4):
                                sc = scp.tile([128, 512], F32, tag="sc",
                                              name=f"sc_{qb}_{kt}_{g}_{q}")
                                nc.tensor.matmul(
                                    out=sc,
                                    lhsT=kT_sb[32 * q : 32 * q + 32,
                                               g * S + kt * 128 : g * S + kt * 128 + 128],
                                    rhs=qT_sb[32 * q : 32 * q + 32,
                                              g * S + qb * 512 : g * S + qb * 512 + 512],
                                    start=True, stop=True,
                                    tile_position=(32 * q, 0),
                                )
                                edst = eT[:, q * 512 : q * 512 + 512]
                                if (hidx % 16) in DVE_SLOTS:
                                    nc.vector.tensor_scalar(
                                        out=edst.bitcast(mybir.dt.int16),
                                        in0=sc, scalar1=A16, scalar2=B16,
                                        op0=MUL, op1=ADD,
                                    )
                                else:
                                    nc.scalar.activation(
                                        out=edst, in_=sc, func=EXP, scale=SCALE,
                                    )
                                hidx += 1
                            if pending is not None:
                                emit_ctx(*pending)
                            pending = (g, kt, eT)
                            # keep the PE stream gapless; extra budget while
                            # the exp pipeline fills at the very start and
                            # around q-block boundaries
                            if qb == 0 and kt == 0:
                                for _ in range(5):
                                    filler(512)
                            elif kt in (15, 0, 1):
                                filler(512)
                                filler(256)
                            else:
                                filler(192)
                                filler(192)
                    emit_ctx(*pending)
                    pending_evict = (qb, ctx_ps, rs_ps)

                # tail: evict + normalize the last q-block
                pqb, pctx, prs = pending_evict
                pstg, prsg = evict_qb(pqb, pctx, prs)
                rcb = norm_a(pqb, pstg, prsg)
                norm_b(pqb, pstg, rcb)

            # ---- phase 4: out = ctxT^T @ w_out + b_out (K=1 ones-row
            #      matmul adds the bias) ----
            with (
                tc.tile_pool(name="op", bufs=4, space="PSUM") as op,
                tc.tile_pool(name="ev", bufs=4) as ev,
            ):
                for st in range(16):
                    ps = op.tile([128, 256], F32, tag="op")
                    nc.tensor.matmul(
                        out=ps, lhsT=ones1_sb[0:1, 0:128], rhs=bo_sb[0:1, :],
                        start=True, stop=False,
                    )
                    for g in range(2):
                        nc.tensor.matmul(
                            out=ps,
                            lhsT=ctxT_sb[g][:, st * 128 : st * 128 + 128],
                            rhs=wo_sb[:, g * 256 : g * 256 + 256],
                            start=False, stop=(g == 1),
                        )
                    ot = ev.tile([128, 256], F32, tag="ot")
                    if st % 2 == 0:
                        nc.scalar.copy(out=ot, in_=ps)
                    else:
                        nc.vector.tensor_copy(out=ot, in_=ps)
                    nc.sync.dma_start(
                        out=out[st * 128 : st * 128 + 128, :], in_=ot
                    )
    if legalize:
        _legalize_sync_waits(nc)
    return nc


_NC_CACHE = None


def _get_nc():
    global _NC_CACHE
    if _NC_CACHE is None:
        _NC_CACHE = _build_nc()
    return _NC_CACHE


def _ks_layout(a, nk, cols):
    """[nk*128, cols] -> [128, nk*cols] with [p, k*cols+c] = a[k*128+p, c]."""
    return np.ascontiguousarray(
        a.reshape(nk, 128, cols).transpose(1, 0, 2).reshape(128, nk * cols)
    )


def _prep_in_maps(x, w_qkv, b_qkv, w_out, b_out):
    x = np.asarray(x, dtype=np.float32)
    w_qkv = np.asarray(w_qkv, dtype=np.float32)
    b_qkv = np.asarray(b_qkv, dtype=np.float32)
    w_out = np.asarray(w_out, dtype=np.float32)
    b_out = np.asarray(b_out, dtype=np.float32)

    # shared (per-core identical) weight layouts
    wqk_l = _ks_layout(w_qkv[:, : 2 * H], 2, 512).astype(NPBF16)
    # v weights in natural head order (32-wide slots)
    wv_l = _ks_layout(w_qkv[:, 2 * H :], 2, 256).astype(NPBF16)
    # out projection: ctxT is packed [(head h)*32 + d] so w_out needs no
    # permutation, only the K-split layout
    wo_l = _ks_layout(w_out, 2, 256).astype(NPBF16)

    shared = {
        "wqk": wqk_l,
        "wv": wv_l,
        "bv": b_qkv[2 * H :].reshape(1, H).astype(NPBF16),
        "wo": wo_l,
        "bo": b_out.reshape(1, H).astype(NPBF16),
        "bqkc": np.ascontiguousarray(
            b_qkv[: 2 * H].astype(np.float32).reshape(4, 128).T
        ),
        "ones": np.ones((1, 512), NPBF16),
    }
    in_maps = []
    for b in range(B):
        xt = _ks_layout(np.ascontiguousarray(x[b].T), 2, S).astype(NPBF16)
        in_maps.append({"xt": xt, **shared})
    return in_maps


def kernel(x, w_qkv, b_qkv, w_out, b_out):
    in_maps = _prep_in_maps(x, w_qkv, b_qkv, w_out, b_out)
    nc = _get_nc()
    res = run_bass_kernel_spmd(nc, in_maps, list(range(N_CORES)), **TRACE_OPTS)
    global LAST_RESULT
    LAST_RESULT = res
    return np.stack([res.results[b]["out"] for b in range(B)], axis=0)
